# revision 13
# baseline (speedup 1.0000x reference)
"""Trainium2 Bass kernel for nn_BilinearFullSymLoss (v4).

Per-sample math (validated against reference in fp64):
  delta(i,j) = wa0*G(i,j) + wa1*G(i+1,j) + wb0*bc(i,j) + wb1*bc(i+1,j)
  bc(i,j)    = cb0*Wn(i,j) + cb1*Wn(i,j+1),  Wn(i,j) = G(i+rb, j+cb)
  pos: wa=(1,0),         wb=(-(1-fy),-fy), rb=dy1,   cb=dx1, valid j in [0,W-dx1-1)
  neg: wa=(-fy,-(1-fy)), wb=(1,0),         rb=dy1+1, cb=dx1, valid j in [-dx1,W)
  loss = m^2 * sum(valid delta^2) / (rows*cols)

Device plan per core (4 samples), all heavy traffic fp16:
- Pool SWDGE casting DMAs load each half image (both channels, f32->f16);
  half-granular loads let sample 0's pipeline start ~1.5MB earlier
- DVE: G = a*g0 + b*g1 per half (ts, ts, tt; fp16 fast modes); an 8-row
  overlap strip (rows 256..263) is recomputed separately so the lower
  window read does not wait for the upper half
- G goes to per-sample lower/upper DRAM scratch tensors; TWO dynamic-offset
  window reads win[p,q,0:W+1] = Gd[rb*W+cb + r*W + j] provide the row and
  column shift in one offset (loaded into an SP register); the upper
  scratch tail is zeroed so no NaNs can reach valid columns
- PE accumulates delta per 128-row block in PSUM with host-built banded
  lhsT:  psd[:,q,:] = mA@G[:,q,:] + mB0@win[:,q,0:W] + mB1@win[:,q,1:W+1]
  mA = wa0*I + wa1*sub, mB0 = cb0*(wb0*I + wb1*sub), mB1 = cb1*(same band),
  all with out-row 127 zeroed: seam rows 127/255/383/511 are added back
  exactly on the host from the f32 grid, so no cross-seam matmuls exist
- ACT squares each 128-row block right after its three matmuls; the
  ivalid-weighted column-sum matmul for block q is emitted after block
  q+1's matmuls, so PE never waits on ACT (fine-grained software pipeline)
- DVE copies [1,W] per sample to SBUF; per-sample DMA writes the output.
Host: sums the valid column range, adds seam rows, scales, means.
"""

import sys

sys.path.insert(0, "/opt/trn_rl_repo")

import numpy as np

import concourse.bass as bass
import concourse.tile as tile
from concourse import mybir
from concourse.bass_utils import run_bass_kernel_spmd

H = 512
W = 512
P = 128
Q = H // P
NS = 4
NCORES = 8
WLEN = W + 1
RPAD = 8
RL = (2 * P + RPAD) * W   # lower scratch: rows 0..255 + 8-row overlap strip
RH = (2 * P + RPAD) * W   # upper scratch: rows 256..511 + zeroed tail

F32 = mybir.dt.float32
F16 = mybir.dt.float16
I32 = mybir.dt.int32

NPF = 2  # a, b
COL_A, COL_B = range(NPF)

_CACHE = {}


def _split_multiwaits(nc):
    """The staged walrus accepts one sync wait per instruction; hoist extras
    onto single-wait NoOps."""
    n = 0
    for fn in nc.m.functions:
        for bb in fn.blocks:
            newlist = []
            for ins in bb.instructions:
                si = ins.sync_info
                if si is not None and si.on_wait is not None and len(si.on_wait) > 1:
                    waits = list(si.on_wait)
                    for w in waits[:-1]:
                        n += 1
                        newlist.append(mybir.InstNoOp(
                            name=f"WSPLIT-{n}-{ins.name}", opcode="NoOp",
                            engine=ins.engine,
                            sync_info=mybir.SyncInfo(on_wait=[w], on_update=[])))
                    ins.sync_info = mybir.SyncInfo(
                        on_wait=[waits[-1]], on_update=list(si.on_update))
                newlist.append(ins)
            bb.instructions = newlist
    return n


def _build_program():
    nc = bass.Bass("TRN2", target_bir_lowering=False, debug=False)

    g = nc.dram_tensor("g", [NS, 2, H, W], F32, kind="ExternalInput")
    pf = nc.dram_tensor("pf", [P, NS * NPF], F32, kind="ExternalInput")
    pi = nc.dram_tensor("pi", [1, NS], I32, kind="ExternalInput")
    iv = nc.dram_tensor("iv", [P, NS * Q], F16, kind="ExternalInput")
    mats = nc.dram_tensor("mats", [P, NS * 3 * P], F16, kind="ExternalInput")
    out = nc.dram_tensor("out", [NS, W], F32, kind="ExternalOutput")
    gdl = [nc.dram_tensor(f"gdl{s}", [RL, 1], F16) for s in range(NS)]
    gdh = [nc.dram_tensor(f"gdh{s}", [RH, 1], F16) for s in range(NS)]

    def _in_full(s):
        # both channels, all rows: (c, q) merges into one contiguous dim
        return bass.AP(tensor=g, offset=s * 2 * H * W,
                       ap=[[W, P], [P * W, 2 * Q], [1, W]])

    with tile.TileContext(nc) as tc:
        with (
            tc.tile_pool(name="consts", bufs=1) as consts,
            tc.tile_pool(name="gh", bufs=2) as ghp,
            tc.tile_pool(name="win", bufs=3) as winp,
            tc.tile_pool(name="work", bufs=3) as work,
            tc.tile_pool(name="psd", bufs=2, space="PSUM") as psdp,
        ):
            # sample-0 load first in line for the DMA engines
            ghs = []
            gh0 = ghp.tile([P, 2 * Q, W], F16, tag="gh", name="gh_0")
            nc.gpsimd.dma_start(gh0[:], _in_full(0))
            ghs.append(gh0)

            pfsb = consts.tile([P, NS * NPF], F32)
            nc.sync.dma_start(pfsb[:], pf[:])
            pisb = consts.tile([1, NS], I32)
            nc.sync.dma_start(pisb[:], pi[:])
            ivsb = consts.tile([P, NS * Q], F16)
            nc.sync.dma_start(ivsb[:], iv[:])
            matsb = consts.tile([P, NS * 3 * P], F16)
            nc.sync.dma_start(matsb[:], mats[:])
            osb = consts.tile([1, NS * W], F32)

            # zero the upper-scratch tails (window overreach past row 511)
            zp = consts.tile([P, RPAD * W // P], F16)
            nc.vector.memset(zp[:], 0.0)
            for s in range(NS):
                nc.sync.dma_start(
                    bass.AP(tensor=gdh[s], offset=2 * P * W,
                            ap=[[RPAD * W // P, P], [1, RPAD * W // P]]),
                    zp[:])

            # remaining input loads (Pool queue; all ready at t=0)
            for s in range(1, NS):
                t = ghp.tile([P, 2 * Q, W], F16, tag="gh", name=f"gh_{s}")
                nc.gpsimd.dma_start(t[:], _in_full(s))
                ghs.append(t)

            state = {}

            def emit_front(s):
                """combine -> scratch writes -> window reads for sample s."""
                pcol = lambda c: pfsb[:, s * NPF + c: s * NPF + c + 1]
                gsb = work.tile([P, Q, W], F16, tag="G", name=f"G_{s}")
                off = nc.values_load(pisb[0:1, s: s + 1],
                                     engines=(mybir.EngineType.SP,),
                                     skip_runtime_bounds_check=True)
                gh = ghs[s]

                # lower-half combine -> lower scratch write
                t0 = work.tile([P, 2, W], F16, tag="t00", name=f"t00_{s}")
                nc.vector.tensor_scalar(
                    out=t0[:], in0=gh[:, 0:2, :], scalar1=pcol(COL_A),
                    scalar2=None, op0=mybir.AluOpType.mult)
                t1 = work.tile([P, 2, W], F16, tag="t10", name=f"t10_{s}")
                nc.vector.tensor_scalar(
                    out=t1[:], in0=gh[:, Q:Q + 2, :], scalar1=pcol(COL_B),
                    scalar2=None, op0=mybir.AluOpType.mult)
                nc.vector.tensor_tensor(out=gsb[:, 0:2, :], in0=t0[:],
                                        in1=t1[:], op=mybir.AluOpType.add)
                nc.sync.dma_start(
                    bass.AP(tensor=gdl[s], offset=0,
                            ap=[[W, P], [P * W, 2], [1, W]]),
                    gsb[:, 0:2, :])

                # 8-row overlap strip (rows 256..263) from the upper load,
                # before the upper combine so the lower window isn't gated
                # on it
                tx0 = work.tile([RPAD, 1, W], F16, tag="tx0", name=f"tx0_{s}")
                nc.vector.tensor_scalar(
                    out=tx0[:], in0=gh[0:RPAD, 2:3, :],
                    scalar1=pfsb[0:RPAD, s * NPF + COL_A: s * NPF + COL_A + 1],
                    scalar2=None, op0=mybir.AluOpType.mult)
                tx1 = work.tile([RPAD, 1, W], F16, tag="tx1", name=f"tx1_{s}")
                nc.vector.tensor_scalar(
                    out=tx1[:], in0=gh[0:RPAD, Q + 2:Q + 3, :],
                    scalar1=pfsb[0:RPAD, s * NPF + COL_B: s * NPF + COL_B + 1],
                    scalar2=None, op0=mybir.AluOpType.mult)
                txs = work.tile([RPAD, 1, W], F16, tag="txs", name=f"txs_{s}")
                nc.vector.tensor_tensor(out=txs[:], in0=tx0[:], in1=tx1[:],
                                        op=mybir.AluOpType.add)
                nc.sync.dma_start(
                    bass.AP(tensor=gdl[s], offset=2 * P * W,
                            ap=[[W, RPAD], [1, W]]),
                    txs[:])
                winL = winp.tile([P, 2, WLEN], F16, tag="win0",
                                 name=f"win0_{s}")
                nc.sync.dma_start(
                    winL[:], bass.AP(tensor=gdl[s], offset=off,
                                     ap=[[W, P], [P * W, 2], [1, WLEN]]))

                # upper-half combine -> upper scratch write -> upper window
                t0h = work.tile([P, 2, W], F16, tag="t01", name=f"t01_{s}")
                nc.vector.tensor_scalar(
                    out=t0h[:], in0=gh[:, 2:4, :], scalar1=pcol(COL_A),
                    scalar2=None, op0=mybir.AluOpType.mult)
                t1h = work.tile([P, 2, W], F16, tag="t11", name=f"t11_{s}")
                nc.vector.tensor_scalar(
                    out=t1h[:], in0=gh[:, Q + 2:Q + 4, :], scalar1=pcol(COL_B),
                    scalar2=None, op0=mybir.AluOpType.mult)
                nc.vector.tensor_tensor(out=gsb[:, 2:4, :], in0=t0h[:],
                                        in1=t1h[:], op=mybir.AluOpType.add)
                nc.sync.dma_start(
                    bass.AP(tensor=gdh[s], offset=0,
                            ap=[[W, P], [P * W, 2], [1, W]]),
                    gsb[:, 2:4, :])
                winH = winp.tile([P, 2, WLEN], F16, tag="win1",
                                 name=f"win1_{s}")
                nc.sync.dma_start(
                    winH[:], bass.AP(tensor=gdh[s], offset=off,
                                     ap=[[W, P], [P * W, 2], [1, WLEN]]))
                wins = [winL, winH]

                psd = psdp.tile([P, Q, W], F32)
                sq = work.tile([P, Q, W], F16, tag="sq", name=f"sq_{s}")
                state[s] = dict(gsb=gsb, wins=wins, psd=psd, sq=sq)

            def emit_mms(s, q):
                st = state[s]
                mA = matsb[:, (3 * s) * P:(3 * s + 1) * P]
                mB0 = matsb[:, (3 * s + 1) * P:(3 * s + 2) * P]
                mB1 = matsb[:, (3 * s + 2) * P:(3 * s + 3) * P]
                psd, gsb = st["psd"], st["gsb"]
                win = st["wins"][q // 2]
                qq = q % 2
                nc.tensor.matmul(psd[:, q, :], lhsT=mA, rhs=gsb[:, q, :],
                                 start=True, stop=False)
                nc.tensor.matmul(psd[:, q, :], lhsT=mB0, rhs=win[:, qq, 0:W],
                                 start=False, stop=False)
                nc.tensor.matmul(psd[:, q, :], lhsT=mB1, rhs=win[:, qq, 1:WLEN],
                                 start=False, stop=True)

            def emit_sq(s, q):
                st = state[s]
                nc.scalar.activation(st["sq"][:, q, :], st["psd"][:, q, :],
                                     mybir.ActivationFunctionType.Square)

            def emit_cs(s, q):
                st = state[s]
                ps = st["psd"][0:1, 0, 0:W]
                nc.tensor.matmul(ps, lhsT=ivsb[:, s * Q + q: s * Q + q + 1],
                                 rhs=st["sq"][:, q, :],
                                 start=(q == 0), stop=(q == Q - 1))
                if q == Q - 1:
                    nc.vector.tensor_copy(osb[0:1, s * W:(s + 1) * W], ps)
                    nc.sync.dma_start(out[s: s + 1, :],
                                      osb[0:1, s * W:(s + 1) * W])

            # fine-grained software pipeline over (s, q) blocks: the column
            # sum for a block trails its matmuls by one block so PE never
            # stalls on ACT
            from collections import deque
            pending = deque()
            for s in range(NS):
                emit_front(s)
                for q in range(Q):
                    emit_mms(s, q)
                    if pending:
                        emit_cs(*pending.popleft())
                    emit_sq(s, q)
                    pending.append((s, q))
            while pending:
                emit_cs(*pending.popleft())

    return nc


def _host_params(gt_sym_axis, gd_sym_axis):
    B = gt_sym_axis.shape[0]
    gt = gt_sym_axis.astype(np.float32)
    gds = gd_sym_axis.astype(np.float32)
    prm = []
    for i in range(B):
        sx = gds[i, 0]
        sy = gds[i, 1]
        dx = np.float32(-10.0) * gt[i, 0]
        dy = np.float32(10.0) * gt[i, 1]
        dy1f = np.float32(np.floor(dy))
        dx1f = np.float32(np.floor(dx))
        dy1 = int(dy1f)
        dx1 = int(dx1f)
        fy = np.float32(dy - dy1f)
        fx = np.float32(dx - dx1f)
        pos = bool(dx > 0)
        one = np.float32(1.0)
        zero = np.float32(0.0)
        if pos:
            wa = (one, zero)
            wb = (-(one - fy), -fy)
            rb, cb = dy1, dx1
            jlo, jhi = 0, W - dx1 - 1
        else:
            wa = (-fy, -(one - fy))
            wb = (one, zero)
            rb, cb = dy1 + 1, dx1
            jlo, jhi = -dx1, W
        rows = H - dy1 - 1
        cols = (W - dx1 - 1) if pos else (W + dx1)
        m = max(abs(float(sx)), abs(float(sy)), 1e-30)
        a = np.float32(float(sy) / m)
        b = np.float32(float(sx) / m)
        off = rb * W + cb
        assert 1 <= off and off + (2 * P - 1) * W + WLEN <= RL
        assert 0 <= rb <= RPAD - 2 and -16 <= cb <= 16
        assert 0 <= jlo <= jhi <= W
        prm.append(dict(a=a, b=b, wa=wa, wb=wb, rb=rb, cb=cb,
                        cb0=one - fx, cb1=fx, jlo=jlo, jhi=jhi,
                        rows=rows, cols=cols, scale=m * m))
    return prm


def _band(w0, w1):
    """lhsT[k, m] = w0*d(k==m) + w1*d(k==m+1), out-row 127 zeroed."""
    mat = np.zeros((P, P), np.float16)
    idx = np.arange(P)
    mat[idx, idx] = np.float16(w0)
    mat[idx[1:], idx[:-1]] = np.float16(w1)
    mat[:, P - 1] = np.float16(0.0)
    return mat


def _seam_fix(grid_s, p):
    """Exact fp64 contribution of the device-zeroed rows 127/255/383/511."""
    g0 = grid_s[0].astype(np.float64)
    g1 = grid_s[1].astype(np.float64)
    G = p["a"] * g0 + p["b"] * g1
    Gp = np.vstack([G, np.zeros((RPAD, W))])
    flat = Gp.reshape(-1)
    wa0, wa1 = float(p["wa"][0]), float(p["wa"][1])
    wb0, wb1 = float(p["wb"][0]), float(p["wb"][1])
    cb0, cb1 = float(p["cb0"]), float(p["cb1"])
    base = p["rb"] * W + p["cb"]
    jlo, jhi = p["jlo"], p["jhi"]
    ssq = 0.0
    for r in (127, 255, 383, 511):
        if r >= p["rows"]:
            continue
        w_r = flat[base + r * W: base + r * W + W + 1]
        w_r1 = flat[base + (r + 1) * W: base + (r + 1) * W + W + 1]
        bc_r = cb0 * w_r[0:W] + cb1 * w_r[1:W + 1]
        bc_r1 = cb0 * w_r1[0:W] + cb1 * w_r1[1:W + 1]
        g_r1 = G[r + 1] if r + 1 < H else np.zeros(W)
        d = wa0 * G[r] + wa1 * g_r1 + wb0 * bc_r + wb1 * bc_r1
        ssq += float((d[jlo:jhi] ** 2).sum())
    return ssq


def kernel(grid, gt_sym_axis, gd_sym_axis):
    grid = np.ascontiguousarray(grid, dtype=np.float32)
    B = grid.shape[0]
    assert grid.shape == (B, 2, H, W) and B == NS * NCORES

    if "nc" not in _CACHE:
        nc = _build_program()
        _split_multiwaits(nc)
        _CACHE["nc"] = nc
    nc = _CACHE["nc"]

    prm = _host_params(np.asarray(gt_sym_axis), np.asarray(gd_sym_axis))

    i_of_pq = np.arange(H).reshape(Q, P).T
    in_maps = []
    for c in range(NCORES):
        pfv = np.zeros((P, NS * NPF), np.float32)
        piv = np.zeros((1, NS), np.int32)
        ivv = np.zeros((P, NS * Q), np.float16)
        matv = np.zeros((P, NS * 3 * P), np.float16)
        for s in range(NS):
            p = prm[c * NS + s]
            pfv[:, s * NPF + COL_A] = p["a"]
            pfv[:, s * NPF + COL_B] = p["b"]
            piv[0, s] = p["rb"] * W + p["cb"]
            ivv[:, s * Q:(s + 1) * Q] = (i_of_pq < p["rows"]).astype(np.float16)
            matv[:, (3 * s) * P:(3 * s + 1) * P] = _band(*p["wa"])
            bb = _band(*p["wb"])
            matv[:, (3 * s + 1) * P:(3 * s + 2) * P] = (
                bb * np.float16(p["cb0"]))
            matv[:, (3 * s + 2) * P:(3 * s + 3) * P] = (
                bb * np.float16(p["cb1"]))
        in_maps.append({
            "g": grid[c * NS:(c + 1) * NS],
            "pf": pfv, "pi": piv, "iv": ivv, "mats": matv,
        })

    res = run_bass_kernel_spmd(nc, in_maps, core_ids=list(range(NCORES)))

    losses = np.zeros(B, np.float64)
    for c in range(NCORES):
        o = res.results[c]["out"]
        for s in range(NS):
            p = prm[c * NS + s]
            ssq = float(o[s, p["jlo"]:p["jhi"]].sum(dtype=np.float64))
            ssq += _seam_fix(grid[c * NS + s], p)
            count = float(np.float32(p["rows"] * p["cols"]))
            losses[c * NS + s] = p["scale"] * ssq / count
    return np.float32(losses.mean())


# revision 15
# speedup vs baseline: 1.2307x; 1.2307x over previous
"""Trainium2 Bass kernel for nn_BilinearFullSymLoss (v5).

Per-sample math (validated against reference in fp64):
  delta(i,j) = wa0*G(i,j) + wa1*G(i+1,j) + wb0*bc(i,j) + wb1*bc(i+1,j)
  bc(i,j)    = cb0*Wn(i,j) + cb1*Wn(i,j+1),  Wn(i,j) = G(i+rb, j+cb)
  pos: wa=(1,0),         wb=(-(1-fy),-fy), rb=dy1,   cb=dx1, valid j in [0,W-dx1-1)
  neg: wa=(-fy,-(1-fy)), wb=(1,0),         rb=dy1+1, cb=dx1, valid j in [-dx1,W)
  loss = m^2 * sum(valid delta^2) / (rows*cols)

Device plan per core (4 samples):
- ONE Pool SWDGE casting DMA per sample loads both channels (f32->f16)
- DVE: G = a*g0 + b*g1 (ts, ts, tt per half; fp16 fast modes)
- G is cast-written (f16->fp8e4m3) by a Pool SWDGE DMA into a per-sample
  DRAM scratch whose 8 tail rows are zeroed; a manual semaphore orders the
  SP-side window read behind the Pool-side write (cross-queue DRAM deps
  are not tracked); ONE dynamic-offset window read
  win[p,q,0:W+1] = Gd[rb*W+cb + r*W + j] yields the row+column shift, fp8
- PE accumulates delta per 128-row block in PSUM with host-built banded
  lhsT (fp16) against mixed-precision rhs:
    psd[:,q,:] = mA@G[:,q,:] (fp16) + mB0@win[:,q,0:W] + mB1@win[:,q,1:W+1]
  (fp8 rhs); mA = wa0*I + wa1*sub, mB0/mB1 = cb0/cb1*(wb0*I + wb1*sub),
  all with out-row 127 zeroed: seam rows 127/255/383/511 are added back
  exactly on the host from the f32 grid, so no cross-seam matmuls exist
- ACT squares each 128-row block right after its three matmuls; the
  ivalid-weighted column-sum matmul for block q trails by one block so PE
  never waits on ACT
- DVE copies [1,W] per sample to SBUF; per-sample DMA writes the output.
Host: sums the valid column range, adds seam rows, scales, means.
"""

import sys

sys.path.insert(0, "/opt/trn_rl_repo")

import numpy as np

import concourse.bass as bass
import concourse.tile as tile
from concourse import mybir
from concourse.bass_utils import run_bass_kernel_spmd

H = 512
W = 512
P = 128
Q = H // P
NS = 4
NCORES = 8
WLEN = W + 1
RPAD = 8
R = (H + RPAD) * W

F32 = mybir.dt.float32
F16 = mybir.dt.float16
F8 = mybir.dt.float8e4
I32 = mybir.dt.int32

NPF = 2  # a, b
COL_A, COL_B = range(NPF)

_CACHE = {}


def _split_multiwaits(nc):
    """The staged walrus accepts one sync wait per instruction; hoist extras
    onto single-wait NoOps."""
    n = 0
    for fn in nc.m.functions:
        for bb in fn.blocks:
            newlist = []
            for ins in bb.instructions:
                si = ins.sync_info
                if si is not None and si.on_wait is not None and len(si.on_wait) > 1:
                    waits = list(si.on_wait)
                    for w in waits[:-1]:
                        n += 1
                        newlist.append(mybir.InstNoOp(
                            name=f"WSPLIT-{n}-{ins.name}", opcode="NoOp",
                            engine=ins.engine,
                            sync_info=mybir.SyncInfo(on_wait=[w], on_update=[])))
                    ins.sync_info = mybir.SyncInfo(
                        on_wait=[waits[-1]], on_update=list(si.on_update))
                newlist.append(ins)
            bb.instructions = newlist
    return n


def _build_program():
    nc = bass.Bass("TRN2", target_bir_lowering=False, debug=False)

    g = nc.dram_tensor("g", [NS, 2, H, W], F32, kind="ExternalInput")
    pf = nc.dram_tensor("pf", [P, NS * NPF], F32, kind="ExternalInput")
    pi = nc.dram_tensor("pi", [1, NS], I32, kind="ExternalInput")
    iv = nc.dram_tensor("iv", [P, NS * Q], F16, kind="ExternalInput")
    mats = nc.dram_tensor("mats", [P, NS * 3 * P], F16, kind="ExternalInput")
    out = nc.dram_tensor("out", [NS, W], F32, kind="ExternalOutput")
    gds = [nc.dram_tensor(f"gd{s}", [R, 1], F8) for s in range(NS)]

    def _in_full(s):
        # both channels, all rows: (c, q) merges into one contiguous dim
        return bass.AP(tensor=g, offset=s * 2 * H * W,
                       ap=[[W, P], [P * W, 2 * Q], [1, W]])

    with tile.TileContext(nc) as tc:
        with (
            tc.tile_pool(name="consts", bufs=1) as consts,
            tc.tile_pool(name="gh", bufs=3) as ghp,
            tc.tile_pool(name="win", bufs=3) as winp,
            tc.tile_pool(name="work", bufs=3) as work,
            tc.tile_pool(name="psd", bufs=2, space="PSUM") as psdp,
        ):
            # sample-0 load first in line for the DMA engines
            ghs = []
            gh0 = ghp.tile([P, 2 * Q, W], F16, tag="gh", name="gh_0")
            nc.gpsimd.dma_start(gh0[:], _in_full(0))
            ghs.append(gh0)

            pfsb = consts.tile([P, NS * NPF], F32)
            nc.sync.dma_start(pfsb[:], pf[:])
            pisb = consts.tile([1, NS], I32)
            nc.sync.dma_start(pisb[:], pi[:])
            ivsb = consts.tile([P, NS * Q], F16)
            nc.sync.dma_start(ivsb[:], iv[:])
            matsb = consts.tile([P, NS * 3 * P], F16)
            nc.sync.dma_start(matsb[:], mats[:])
            osb = consts.tile([1, NS * W], F32)

            # zero the scratch tails (window overreach past row 511)
            zp = consts.tile([P, RPAD * W // P], F8)
            nc.vector.memset(zp[:], 0.0)
            for s in range(NS):
                nc.sync.dma_start(
                    bass.AP(tensor=gds[s], offset=H * W,
                            ap=[[RPAD * W // P, P], [1, RPAD * W // P]]),
                    zp[:])

            # remaining input loads (Pool queue)
            for s in range(1, NS):
                t = ghp.tile([P, 2 * Q, W], F16, tag="gh", name=f"gh_{s}")
                nc.gpsimd.dma_start(t[:], _in_full(s))
                ghs.append(t)

            state = {}

            def emit_front(s):
                """combine -> fp8 scratch write -> window read for sample s."""
                pcol = lambda c: pfsb[:, s * NPF + c: s * NPF + c + 1]
                gh = ghs[s]
                gsb = work.tile([P, Q, W], F16, tag="G", name=f"G_{s}")
                for half, lo in ((0, 0), (1, 2)):
                    t0 = work.tile([P, 2, W], F16, tag=f"t0{half}",
                                   name=f"t0{half}_{s}")
                    nc.vector.tensor_scalar(
                        out=t0[:], in0=gh[:, lo:lo + 2, :], scalar1=pcol(COL_A),
                        scalar2=None, op0=mybir.AluOpType.mult)
                    t1 = work.tile([P, 2, W], F16, tag=f"t1{half}",
                                   name=f"t1{half}_{s}")
                    nc.vector.tensor_scalar(
                        out=t1[:], in0=gh[:, Q + lo:Q + lo + 2, :],
                        scalar1=pcol(COL_B),
                        scalar2=None, op0=mybir.AluOpType.mult)
                    nc.vector.tensor_tensor(out=gsb[:, lo:lo + 2, :],
                                            in0=t0[:], in1=t1[:],
                                            op=mybir.AluOpType.add)

                # fp8 cast write (Pool SWDGE); the window read follows on
                # the same in-order Pool queue, so no cross-queue ordering
                # is needed
                nc.gpsimd.dma_start(
                    bass.AP(tensor=gds[s], offset=0,
                            ap=[[W, P], [P * W, Q], [1, W]]),
                    gsb[:])
                off = nc.values_load(pisb[0:1, s: s + 1],
                                     engines=(mybir.EngineType.Pool,),
                                     skip_runtime_bounds_check=True)
                win = winp.tile([P, Q, WLEN], F8, tag="win", name=f"win_{s}")
                nc.gpsimd.dma_start(
                    win[:], bass.AP(tensor=gds[s], offset=off,
                                    ap=[[W, P], [P * W, Q], [1, WLEN]]))

                psd = psdp.tile([P, Q, W], F32)
                sq = work.tile([P, Q, W], F16, tag="sq", name=f"sq_{s}")
                state[s] = dict(gsb=gsb, win=win, psd=psd, sq=sq)

            def emit_mms(s, q):
                st = state[s]
                mA = matsb[:, (3 * s) * P:(3 * s + 1) * P]
                mB0 = matsb[:, (3 * s + 1) * P:(3 * s + 2) * P]
                mB1 = matsb[:, (3 * s + 2) * P:(3 * s + 3) * P]
                psd, gsb, win = st["psd"], st["gsb"], st["win"]
                nc.tensor.matmul(psd[:, q, :], lhsT=mA, rhs=gsb[:, q, :],
                                 start=True, stop=False)
                nc.tensor.matmul(psd[:, q, :], lhsT=mB0, rhs=win[:, q, 0:W],
                                 start=False, stop=False)
                nc.tensor.matmul(psd[:, q, :], lhsT=mB1, rhs=win[:, q, 1:WLEN],
                                 start=False, stop=True)

            def emit_sq(s, q):
                st = state[s]
                nc.scalar.activation(st["sq"][:, q, :], st["psd"][:, q, :],
                                     mybir.ActivationFunctionType.Square)

            def emit_cs(s, q):
                st = state[s]
                ps = st["psd"][0:1, 0, 0:W]
                nc.tensor.matmul(ps, lhsT=ivsb[:, s * Q + q: s * Q + q + 1],
                                 rhs=st["sq"][:, q, :],
                                 start=(q == 0), stop=(q == Q - 1))
                if q == Q - 1:
                    nc.vector.tensor_copy(osb[0:1, s * W:(s + 1) * W], ps)
                    nc.sync.dma_start(out[s: s + 1, :],
                                      osb[0:1, s * W:(s + 1) * W])

            # fine-grained software pipeline over (s, q) blocks
            from collections import deque
            pending = deque()
            for s in range(NS):
                emit_front(s)
                for q in range(Q):
                    emit_mms(s, q)
                    if pending:
                        emit_cs(*pending.popleft())
                    emit_sq(s, q)
                    pending.append((s, q))
            while pending:
                emit_cs(*pending.popleft())

    return nc


def _host_params(gt_sym_axis, gd_sym_axis):
    B = gt_sym_axis.shape[0]
    gt = gt_sym_axis.astype(np.float32)
    gds = gd_sym_axis.astype(np.float32)
    prm = []
    for i in range(B):
        sx = gds[i, 0]
        sy = gds[i, 1]
        dx = np.float32(-10.0) * gt[i, 0]
        dy = np.float32(10.0) * gt[i, 1]
        dy1f = np.float32(np.floor(dy))
        dx1f = np.float32(np.floor(dx))
        dy1 = int(dy1f)
        dx1 = int(dx1f)
        fy = np.float32(dy - dy1f)
        fx = np.float32(dx - dx1f)
        pos = bool(dx > 0)
        one = np.float32(1.0)
        zero = np.float32(0.0)
        if pos:
            wa = (one, zero)
            wb = (-(one - fy), -fy)
            rb, cb = dy1, dx1
            jlo, jhi = 0, W - dx1 - 1
        else:
            wa = (-fy, -(one - fy))
            wb = (one, zero)
            rb, cb = dy1 + 1, dx1
            jlo, jhi = -dx1, W
        rows = H - dy1 - 1
        cols = (W - dx1 - 1) if pos else (W + dx1)
        m = max(abs(float(sx)), abs(float(sy)), 1e-30)
        a = np.float32(float(sy) / m)
        b = np.float32(float(sx) / m)
        off = rb * W + cb
        assert 1 <= off and off + (H - 1) * W + WLEN <= R
        assert 0 <= rb <= RPAD - 2 and -16 <= cb <= 16
        assert 0 <= jlo <= jhi <= W
        prm.append(dict(a=a, b=b, wa=wa, wb=wb, rb=rb, cb=cb,
                        cb0=one - fx, cb1=fx, jlo=jlo, jhi=jhi,
                        rows=rows, cols=cols, scale=m * m))
    return prm


def _band(w0, w1):
    """lhsT[k, m] = w0*d(k==m) + w1*d(k==m+1), out-row 127 zeroed."""
    mat = np.zeros((P, P), np.float16)
    idx = np.arange(P)
    mat[idx, idx] = np.float16(w0)
    mat[idx[1:], idx[:-1]] = np.float16(w1)
    mat[:, P - 1] = np.float16(0.0)
    return mat


def _seam_fix(grid_s, p):
    """Exact fp64 contribution of the device-zeroed rows 127/255/383/511."""
    g0 = grid_s[0].astype(np.float64)
    g1 = grid_s[1].astype(np.float64)
    G = p["a"] * g0 + p["b"] * g1
    Gp = np.vstack([G, np.zeros((RPAD, W))])
    flat = Gp.reshape(-1)
    wa0, wa1 = float(p["wa"][0]), float(p["wa"][1])
    wb0, wb1 = float(p["wb"][0]), float(p["wb"][1])
    cb0, cb1 = float(p["cb0"]), float(p["cb1"])
    base = p["rb"] * W + p["cb"]
    jlo, jhi = p["jlo"], p["jhi"]
    ssq = 0.0
    for r in (127, 255, 383, 511):
        if r >= p["rows"]:
            continue
        w_r = flat[base + r * W: base + r * W + W + 1]
        w_r1 = flat[base + (r + 1) * W: base + (r + 1) * W + W + 1]
        bc_r = cb0 * w_r[0:W] + cb1 * w_r[1:W + 1]
        bc_r1 = cb0 * w_r1[0:W] + cb1 * w_r1[1:W + 1]
        g_r1 = G[r + 1] if r + 1 < H else np.zeros(W)
        d = wa0 * G[r] + wa1 * g_r1 + wb0 * bc_r + wb1 * bc_r1
        ssq += float((d[jlo:jhi] ** 2).sum())
    return ssq


def kernel(grid, gt_sym_axis, gd_sym_axis):
    grid = np.ascontiguousarray(grid, dtype=np.float32)
    B = grid.shape[0]
    assert grid.shape == (B, 2, H, W) and B == NS * NCORES

    if "nc" not in _CACHE:
        nc = _build_program()
        _split_multiwaits(nc)
        _CACHE["nc"] = nc
    nc = _CACHE["nc"]

    prm = _host_params(np.asarray(gt_sym_axis), np.asarray(gd_sym_axis))

    i_of_pq = np.arange(H).reshape(Q, P).T
    in_maps = []
    for c in range(NCORES):
        pfv = np.zeros((P, NS * NPF), np.float32)
        piv = np.zeros((1, NS), np.int32)
        ivv = np.zeros((P, NS * Q), np.float16)
        matv = np.zeros((P, NS * 3 * P), np.float16)
        for s in range(NS):
            p = prm[c * NS + s]
            pfv[:, s * NPF + COL_A] = p["a"]
            pfv[:, s * NPF + COL_B] = p["b"]
            piv[0, s] = p["rb"] * W + p["cb"]
            ivv[:, s * Q:(s + 1) * Q] = (i_of_pq < p["rows"]).astype(np.float16)
            matv[:, (3 * s) * P:(3 * s + 1) * P] = _band(*p["wa"])
            bb = _band(*p["wb"])
            matv[:, (3 * s + 1) * P:(3 * s + 2) * P] = (
                bb * np.float16(p["cb0"]))
            matv[:, (3 * s + 2) * P:(3 * s + 3) * P] = (
                bb * np.float16(p["cb1"]))
        in_maps.append({
            "g": grid[c * NS:(c + 1) * NS],
            "pf": pfv, "pi": piv, "iv": ivv, "mats": matv,
        })

    res = run_bass_kernel_spmd(nc, in_maps, core_ids=list(range(NCORES)))

    losses = np.zeros(B, np.float64)
    for c in range(NCORES):
        o = res.results[c]["out"]
        for s in range(NS):
            p = prm[c * NS + s]
            ssq = float(o[s, p["jlo"]:p["jhi"]].sum(dtype=np.float64))
            ssq += _seam_fix(grid[c * NS + s], p)
            count = float(np.float32(p["rows"] * p["cols"]))
            losses[c * NS + s] = p["scale"] * ssq / count
    return np.float32(losses.mean())


# revision 16
# speedup vs baseline: 1.2571x; 1.0215x over previous
"""Trainium2 Bass kernel for nn_BilinearFullSymLoss (v5).

Per-sample math (validated against reference in fp64):
  delta(i,j) = wa0*G(i,j) + wa1*G(i+1,j) + wb0*bc(i,j) + wb1*bc(i+1,j)
  bc(i,j)    = cb0*Wn(i,j) + cb1*Wn(i,j+1),  Wn(i,j) = G(i+rb, j+cb)
  pos: wa=(1,0),         wb=(-(1-fy),-fy), rb=dy1,   cb=dx1, valid j in [0,W-dx1-1)
  neg: wa=(-fy,-(1-fy)), wb=(1,0),         rb=dy1+1, cb=dx1, valid j in [-dx1,W)
  loss = m^2 * sum(valid delta^2) / (rows*cols)

Device plan per core (4 samples):
- ONE Pool SWDGE casting DMA per sample loads both channels (f32->f16)
- DVE: G = a*g0 + b*g1 (ts, ts, tt per half; fp16 fast modes)
- G is cast-written (f16->fp8e4m3) by a Pool SWDGE DMA into a per-sample
  DRAM scratch whose 8 tail rows are zeroed; a manual semaphore orders the
  SP-side window read behind the Pool-side write (cross-queue DRAM deps
  are not tracked); ONE dynamic-offset window read
  win[p,q,0:W+1] = Gd[rb*W+cb + r*W + j] yields the row+column shift, fp8
- PE accumulates delta per 128-row block in PSUM with host-built banded
  lhsT (fp16) against mixed-precision rhs:
    psd[:,q,:] = mA@G[:,q,:] (fp16) + mB0@win[:,q,0:W] + mB1@win[:,q,1:W+1]
  (fp8 rhs); mA = wa0*I + wa1*sub, mB0/mB1 = cb0/cb1*(wb0*I + wb1*sub),
  all with out-row 127 zeroed: seam rows 127/255/383/511 are added back
  exactly on the host from the f32 grid, so no cross-seam matmuls exist
- ACT squares each 128-row block right after its three matmuls; the
  ivalid-weighted column-sum matmul for block q trails by one block so PE
  never waits on ACT
- DVE copies [1,W] per sample to SBUF; per-sample DMA writes the output.
Host: sums the valid column range, adds seam rows, scales, means.
"""

import sys

sys.path.insert(0, "/opt/trn_rl_repo")

import numpy as np

import concourse.bass as bass
import concourse.tile as tile
from concourse import mybir
from concourse.bass_utils import run_bass_kernel_spmd

H = 512
W = 512
P = 128
Q = H // P
NS = 4
NCORES = 8
WLEN = W + 1
RPAD = 8
R = (H + RPAD) * W

F32 = mybir.dt.float32
F16 = mybir.dt.float16
F8 = mybir.dt.float8e4
I32 = mybir.dt.int32

NPF = 2  # a, b
COL_A, COL_B = range(NPF)

_CACHE = {}


def _split_multiwaits(nc):
    """The staged walrus accepts one sync wait per instruction; hoist extras
    onto single-wait NoOps."""
    n = 0
    for fn in nc.m.functions:
        for bb in fn.blocks:
            newlist = []
            for ins in bb.instructions:
                si = ins.sync_info
                if si is not None and si.on_wait is not None and len(si.on_wait) > 1:
                    waits = list(si.on_wait)
                    for w in waits[:-1]:
                        n += 1
                        newlist.append(mybir.InstNoOp(
                            name=f"WSPLIT-{n}-{ins.name}", opcode="NoOp",
                            engine=ins.engine,
                            sync_info=mybir.SyncInfo(on_wait=[w], on_update=[])))
                    ins.sync_info = mybir.SyncInfo(
                        on_wait=[waits[-1]], on_update=list(si.on_update))
                newlist.append(ins)
            bb.instructions = newlist
    return n


def _build_program():
    nc = bass.Bass("TRN2", target_bir_lowering=False, debug=False)

    g = nc.dram_tensor("g", [NS, 2, H, W], F32, kind="ExternalInput")
    pf = nc.dram_tensor("pf", [P, NS * NPF], F32, kind="ExternalInput")
    pi = nc.dram_tensor("pi", [1, NS], I32, kind="ExternalInput")
    iv = nc.dram_tensor("iv", [P, NS * Q], F16, kind="ExternalInput")
    mats = nc.dram_tensor("mats", [P, NS * 3 * P], F16, kind="ExternalInput")
    out = nc.dram_tensor("out", [NS, W], F32, kind="ExternalOutput")
    gds = [nc.dram_tensor(f"gd{s}", [R, 1], F8) for s in range(NS)]

    def _in_full(s):
        # both channels, all rows: (c, q) merges into one contiguous dim
        return bass.AP(tensor=g, offset=s * 2 * H * W,
                       ap=[[W, P], [P * W, 2 * Q], [1, W]])

    wr_rd_pairs = []

    with tile.TileContext(nc) as tc:
        with (
            tc.tile_pool(name="consts", bufs=1) as consts,
            tc.tile_pool(name="gh", bufs=3) as ghp,
            tc.tile_pool(name="win", bufs=3) as winp,
            tc.tile_pool(name="work", bufs=3) as work,
            tc.tile_pool(name="psd", bufs=2, space="PSUM") as psdp,
        ):
            # sample-0 load first in line for the DMA engines
            ghs = []
            gh0 = ghp.tile([P, 2 * Q, W], F16, tag="gh", name="gh_0")
            nc.gpsimd.dma_start(gh0[:], _in_full(0))
            ghs.append(gh0)

            pfsb = consts.tile([P, NS * NPF], F32)
            nc.sync.dma_start(pfsb[:], pf[:])
            pisb = consts.tile([1, NS], I32)
            nc.sync.dma_start(pisb[:], pi[:])
            ivsb = consts.tile([P, NS * Q], F16)
            nc.sync.dma_start(ivsb[:], iv[:])
            matsb = consts.tile([P, NS * 3 * P], F16)
            nc.sync.dma_start(matsb[:], mats[:])
            osb = consts.tile([1, NS * W], F32)

            # zero the scratch tails (window overreach past row 511)
            zp = consts.tile([P, RPAD * W // P], F8)
            nc.vector.memset(zp[:], 0.0)
            for s in range(NS):
                nc.sync.dma_start(
                    bass.AP(tensor=gds[s], offset=H * W,
                            ap=[[RPAD * W // P, P], [1, RPAD * W // P]]),
                    zp[:])

            # remaining input loads (Pool queue)
            for s in range(1, NS):
                t = ghp.tile([P, 2 * Q, W], F16, tag="gh", name=f"gh_{s}")
                nc.gpsimd.dma_start(t[:], _in_full(s))
                ghs.append(t)

            state = {}

            def emit_front(s):
                """combine -> fp8 scratch write -> window read for sample s."""
                pcol = lambda c: pfsb[:, s * NPF + c: s * NPF + c + 1]
                gh = ghs[s]
                gsb = work.tile([P, Q, W], F16, tag="G", name=f"G_{s}")
                for half, lo in ((0, 0), (1, 2)):
                    t0 = work.tile([P, 2, W], F16, tag=f"t0{half}",
                                   name=f"t0{half}_{s}")
                    nc.vector.tensor_scalar(
                        out=t0[:], in0=gh[:, lo:lo + 2, :], scalar1=pcol(COL_A),
                        scalar2=None, op0=mybir.AluOpType.mult)
                    t1 = work.tile([P, 2, W], F16, tag=f"t1{half}",
                                   name=f"t1{half}_{s}")
                    nc.vector.tensor_scalar(
                        out=t1[:], in0=gh[:, Q + lo:Q + lo + 2, :],
                        scalar1=pcol(COL_B),
                        scalar2=None, op0=mybir.AluOpType.mult)
                    nc.vector.tensor_tensor(out=gsb[:, lo:lo + 2, :],
                                            in0=t0[:], in1=t1[:],
                                            op=mybir.AluOpType.add)

                # fp8 cast write (Pool SWDGE); the SP-side window read is
                # ordered behind it by a post-build pass that makes it wait
                # on the write's own queue-completion semaphore
                wr = nc.gpsimd.dma_start(
                    bass.AP(tensor=gds[s], offset=0,
                            ap=[[W, P], [P * W, Q], [1, W]]),
                    gsb[:])
                off = nc.values_load(pisb[0:1, s: s + 1],
                                     engines=(mybir.EngineType.SP,),
                                     skip_runtime_bounds_check=True)
                win = winp.tile([P, Q, WLEN], F8, tag="win", name=f"win_{s}")
                rd = nc.sync.dma_start(
                    win[:], bass.AP(tensor=gds[s], offset=off,
                                    ap=[[W, P], [P * W, Q], [1, WLEN]]))
                wr_rd_pairs.append((wr.ins.name, rd.ins.name))

                psd = psdp.tile([P, Q, W], F32)
                sq = work.tile([P, Q, W], F16, tag="sq", name=f"sq_{s}")
                state[s] = dict(gsb=gsb, win=win, psd=psd, sq=sq)

            def emit_mms(s, q):
                st = state[s]
                mA = matsb[:, (3 * s) * P:(3 * s + 1) * P]
                mB0 = matsb[:, (3 * s + 1) * P:(3 * s + 2) * P]
                mB1 = matsb[:, (3 * s + 2) * P:(3 * s + 3) * P]
                psd, gsb, win = st["psd"], st["gsb"], st["win"]
                nc.tensor.matmul(psd[:, q, :], lhsT=mA, rhs=gsb[:, q, :],
                                 start=True, stop=False)
                nc.tensor.matmul(psd[:, q, :], lhsT=mB0, rhs=win[:, q, 0:W],
                                 start=False, stop=False)
                nc.tensor.matmul(psd[:, q, :], lhsT=mB1, rhs=win[:, q, 1:WLEN],
                                 start=False, stop=True)

            def emit_sq(s, q):
                st = state[s]
                nc.scalar.activation(st["sq"][:, q, :], st["psd"][:, q, :],
                                     mybir.ActivationFunctionType.Square)

            def emit_cs(s, q):
                st = state[s]
                ps = st["psd"][0:1, 0, 0:W]
                nc.tensor.matmul(ps, lhsT=ivsb[:, s * Q + q: s * Q + q + 1],
                                 rhs=st["sq"][:, q, :],
                                 start=(q == 0), stop=(q == Q - 1))
                if q == Q - 1:
                    nc.vector.tensor_copy(osb[0:1, s * W:(s + 1) * W], ps)
                    nc.sync.dma_start(out[s: s + 1, :],
                                      osb[0:1, s * W:(s + 1) * W])

            # fine-grained software pipeline over (s, q) blocks
            from collections import deque
            pending = deque()
            for s in range(NS):
                emit_front(s)
                for q in range(Q):
                    emit_mms(s, q)
                    if pending:
                        emit_cs(*pending.popleft())
                    emit_sq(s, q)
                    pending.append((s, q))
            while pending:
                emit_cs(*pending.popleft())

    _order_scratch_reads(nc, wr_rd_pairs)
    return nc


def _order_scratch_reads(nc, wr_rd_pairs):
    """Make each SP window read wait on its Pool scratch write's completion
    (cross-queue DRAM deps aren't auto-tracked). Each DMA updates one of the
    rotating queue sems; the wait value is the cumulative update count of
    that sem up to and including the write."""
    wr_names = {w: r for (w, r) in wr_rd_pairs}
    cum = {}
    waits_for_rd = {}
    for fn in nc.m.functions:
        for bb in fn.blocks:
            for ins in bb.instructions:
                si = ins.sync_info
                if si is None or not si.on_update:
                    continue
                for u in si.on_update:
                    if getattr(u, "sync_type", None) != "semaphore":
                        continue
                    key = u.id
                    cum[key] = cum.get(key, 0) + (u.update_value or 0)
                    if ins.name in wr_names and ins.opcode == "DMACopy":
                        waits_for_rd[wr_names[ins.name]] = (
                            u.id, u.ant_name, cum[key])
    fixed = 0
    for fn in nc.m.functions:
        for bb in fn.blocks:
            for ins in bb.instructions:
                if ins.name in waits_for_rd:
                    sid, sname, val = waits_for_rd[ins.name]
                    si = ins.sync_info or mybir.SyncInfo(on_wait=[],
                                                         on_update=[])
                    si.on_wait = list(si.on_wait or []) + [mybir.SyncWait(
                        sync_type="semaphore", id=sid, ant_name=sname,
                        wait_mode="sem-ge-imm", wait_value=val)]
                    ins.sync_info = si
                    fixed += 1
    assert fixed == len(wr_rd_pairs), (fixed, len(wr_rd_pairs))


def _host_params(gt_sym_axis, gd_sym_axis):
    B = gt_sym_axis.shape[0]
    gt = gt_sym_axis.astype(np.float32)
    gds = gd_sym_axis.astype(np.float32)
    prm = []
    for i in range(B):
        sx = gds[i, 0]
        sy = gds[i, 1]
        dx = np.float32(-10.0) * gt[i, 0]
        dy = np.float32(10.0) * gt[i, 1]
        dy1f = np.float32(np.floor(dy))
        dx1f = np.float32(np.floor(dx))
        dy1 = int(dy1f)
        dx1 = int(dx1f)
        fy = np.float32(dy - dy1f)
        fx = np.float32(dx - dx1f)
        pos = bool(dx > 0)
        one = np.float32(1.0)
        zero = np.float32(0.0)
        if pos:
            wa = (one, zero)
            wb = (-(one - fy), -fy)
            rb, cb = dy1, dx1
            jlo, jhi = 0, W - dx1 - 1
        else:
            wa = (-fy, -(one - fy))
            wb = (one, zero)
            rb, cb = dy1 + 1, dx1
            jlo, jhi = -dx1, W
        rows = H - dy1 - 1
        cols = (W - dx1 - 1) if pos else (W + dx1)
        m = max(abs(float(sx)), abs(float(sy)), 1e-30)
        a = np.float32(float(sy) / m)
        b = np.float32(float(sx) / m)
        off = rb * W + cb
        assert 1 <= off and off + (H - 1) * W + WLEN <= R
        assert 0 <= rb <= RPAD - 2 and -16 <= cb <= 16
        assert 0 <= jlo <= jhi <= W
        prm.append(dict(a=a, b=b, wa=wa, wb=wb, rb=rb, cb=cb,
                        cb0=one - fx, cb1=fx, jlo=jlo, jhi=jhi,
                        rows=rows, cols=cols, scale=m * m))
    return prm


def _band(w0, w1):
    """lhsT[k, m] = w0*d(k==m) + w1*d(k==m+1), out-row 127 zeroed."""
    mat = np.zeros((P, P), np.float16)
    idx = np.arange(P)
    mat[idx, idx] = np.float16(w0)
    mat[idx[1:], idx[:-1]] = np.float16(w1)
    mat[:, P - 1] = np.float16(0.0)
    return mat


def _seam_fix(grid_s, p):
    """Exact fp64 contribution of the device-zeroed rows 127/255/383/511."""
    g0 = grid_s[0].astype(np.float64)
    g1 = grid_s[1].astype(np.float64)
    G = p["a"] * g0 + p["b"] * g1
    Gp = np.vstack([G, np.zeros((RPAD, W))])
    flat = Gp.reshape(-1)
    wa0, wa1 = float(p["wa"][0]), float(p["wa"][1])
    wb0, wb1 = float(p["wb"][0]), float(p["wb"][1])
    cb0, cb1 = float(p["cb0"]), float(p["cb1"])
    base = p["rb"] * W + p["cb"]
    jlo, jhi = p["jlo"], p["jhi"]
    ssq = 0.0
    for r in (127, 255, 383, 511):
        if r >= p["rows"]:
            continue
        w_r = flat[base + r * W: base + r * W + W + 1]
        w_r1 = flat[base + (r + 1) * W: base + (r + 1) * W + W + 1]
        bc_r = cb0 * w_r[0:W] + cb1 * w_r[1:W + 1]
        bc_r1 = cb0 * w_r1[0:W] + cb1 * w_r1[1:W + 1]
        g_r1 = G[r + 1] if r + 1 < H else np.zeros(W)
        d = wa0 * G[r] + wa1 * g_r1 + wb0 * bc_r + wb1 * bc_r1
        ssq += float((d[jlo:jhi] ** 2).sum())
    return ssq


def kernel(grid, gt_sym_axis, gd_sym_axis):
    grid = np.ascontiguousarray(grid, dtype=np.float32)
    B = grid.shape[0]
    assert grid.shape == (B, 2, H, W) and B == NS * NCORES

    if "nc" not in _CACHE:
        nc = _build_program()
        _split_multiwaits(nc)
        _CACHE["nc"] = nc
    nc = _CACHE["nc"]

    prm = _host_params(np.asarray(gt_sym_axis), np.asarray(gd_sym_axis))

    i_of_pq = np.arange(H).reshape(Q, P).T
    in_maps = []
    for c in range(NCORES):
        pfv = np.zeros((P, NS * NPF), np.float32)
        piv = np.zeros((1, NS), np.int32)
        ivv = np.zeros((P, NS * Q), np.float16)
        matv = np.zeros((P, NS * 3 * P), np.float16)
        for s in range(NS):
            p = prm[c * NS + s]
            pfv[:, s * NPF + COL_A] = p["a"]
            pfv[:, s * NPF + COL_B] = p["b"]
            piv[0, s] = p["rb"] * W + p["cb"]
            ivv[:, s * Q:(s + 1) * Q] = (i_of_pq < p["rows"]).astype(np.float16)
            matv[:, (3 * s) * P:(3 * s + 1) * P] = _band(*p["wa"])
            bb = _band(*p["wb"])
            matv[:, (3 * s + 1) * P:(3 * s + 2) * P] = (
                bb * np.float16(p["cb0"]))
            matv[:, (3 * s + 2) * P:(3 * s + 3) * P] = (
                bb * np.float16(p["cb1"]))
        in_maps.append({
            "g": grid[c * NS:(c + 1) * NS],
            "pf": pfv, "pi": piv, "iv": ivv, "mats": matv,
        })

    res = run_bass_kernel_spmd(nc, in_maps, core_ids=list(range(NCORES)))

    losses = np.zeros(B, np.float64)
    for c in range(NCORES):
        o = res.results[c]["out"]
        for s in range(NS):
            p = prm[c * NS + s]
            ssq = float(o[s, p["jlo"]:p["jhi"]].sum(dtype=np.float64))
            ssq += _seam_fix(grid[c * NS + s], p)
            count = float(np.float32(p["rows"] * p["cols"]))
            losses[c * NS + s] = p["scale"] * ssq / count
    return np.float32(losses.mean())


# revision 17
# speedup vs baseline: 1.2744x; 1.0137x over previous
"""Trainium2 Bass kernel for nn_BilinearFullSymLoss (v5).

Per-sample math (validated against reference in fp64):
  delta(i,j) = wa0*G(i,j) + wa1*G(i+1,j) + wb0*bc(i,j) + wb1*bc(i+1,j)
  bc(i,j)    = cb0*Wn(i,j) + cb1*Wn(i,j+1),  Wn(i,j) = G(i+rb, j+cb)
  pos: wa=(1,0),         wb=(-(1-fy),-fy), rb=dy1,   cb=dx1, valid j in [0,W-dx1-1)
  neg: wa=(-fy,-(1-fy)), wb=(1,0),         rb=dy1+1, cb=dx1, valid j in [-dx1,W)
  loss = m^2 * sum(valid delta^2) / (rows*cols)

Device plan per core (4 samples):
- ONE Pool SWDGE casting DMA per sample loads both channels (f32->f16)
- DVE: G = a*g0 + b*g1 (ts, ts, tt per half; fp16 fast modes)
- G is cast-written (f16->fp8e4m3) by a Pool SWDGE DMA into a per-sample
  DRAM scratch whose 8 tail rows are zeroed; a manual semaphore orders the
  SP-side window read behind the Pool-side write (cross-queue DRAM deps
  are not tracked); ONE dynamic-offset window read
  win[p,q,0:W+1] = Gd[rb*W+cb + r*W + j] yields the row+column shift, fp8
- PE accumulates delta per 128-row block in PSUM with host-built banded
  lhsT (fp16) against mixed-precision rhs:
    psd[:,q,:] = mA@G[:,q,:] (fp16) + mB0@win[:,q,0:W] + mB1@win[:,q,1:W+1]
  (fp8 rhs); mA = wa0*I + wa1*sub, mB0/mB1 = cb0/cb1*(wb0*I + wb1*sub),
  all with out-row 127 zeroed: seam rows 127/255/383/511 are added back
  exactly on the host from the f32 grid, so no cross-seam matmuls exist
- ACT squares each 128-row block right after its three matmuls; the
  ivalid-weighted column-sum matmul for block q trails by one block so PE
  never waits on ACT
- DVE copies [1,W] per sample to SBUF; per-sample DMA writes the output.
Host: sums the valid column range, adds seam rows, scales, means.
"""

import sys

sys.path.insert(0, "/opt/trn_rl_repo")

import numpy as np

import concourse.bass as bass
import concourse.tile as tile
from concourse import mybir
from concourse.bass_utils import run_bass_kernel_spmd

H = 512
W = 512
P = 128
Q = H // P
NS = 4
NCORES = 8
WLEN = W + 1
RPAD = 8
R = (H + RPAD) * W

F32 = mybir.dt.float32
F16 = mybir.dt.float16
F8 = mybir.dt.float8e4
I32 = mybir.dt.int32

NPF = 2  # a, b
COL_A, COL_B = range(NPF)

_CACHE = {}


def _split_multiwaits(nc):
    """The staged walrus accepts one sync wait per instruction; hoist extras
    onto single-wait NoOps."""
    n = 0
    for fn in nc.m.functions:
        for bb in fn.blocks:
            newlist = []
            for ins in bb.instructions:
                si = ins.sync_info
                if si is not None and si.on_wait is not None and len(si.on_wait) > 1:
                    waits = list(si.on_wait)
                    for w in waits[:-1]:
                        n += 1
                        newlist.append(mybir.InstNoOp(
                            name=f"WSPLIT-{n}-{ins.name}", opcode="NoOp",
                            engine=ins.engine,
                            sync_info=mybir.SyncInfo(on_wait=[w], on_update=[])))
                    ins.sync_info = mybir.SyncInfo(
                        on_wait=[waits[-1]], on_update=list(si.on_update))
                newlist.append(ins)
            bb.instructions = newlist
    return n


def _build_program():
    nc = bass.Bass("TRN2", target_bir_lowering=False, debug=False)

    g = nc.dram_tensor("g", [NS, 2, H, W], F32, kind="ExternalInput")
    pf = nc.dram_tensor("pf", [P, NS * NPF], F32, kind="ExternalInput")
    pi = nc.dram_tensor("pi", [1, NS], I32, kind="ExternalInput")
    iv = nc.dram_tensor("iv", [P, NS * Q], F16, kind="ExternalInput")
    mats = nc.dram_tensor("mats", [P, NS * 3 * P], F16, kind="ExternalInput")
    out = nc.dram_tensor("out", [NS, W], F32, kind="ExternalOutput")
    gds = [nc.dram_tensor(f"gd{s}", [R, 1], F8) for s in range(NS)]

    def _in_full(s):
        # both channels, all rows: (c, q) merges into one contiguous dim
        return bass.AP(tensor=g, offset=s * 2 * H * W,
                       ap=[[W, P], [P * W, 2 * Q], [1, W]])

    wr_rd_pairs = []

    with tile.TileContext(nc) as tc:
        with (
            tc.tile_pool(name="consts", bufs=1) as consts,
            tc.tile_pool(name="gh", bufs=4) as ghp,
            tc.tile_pool(name="win", bufs=3) as winp,
            tc.tile_pool(name="work", bufs=3) as work,
            tc.tile_pool(name="psd", bufs=2, space="PSUM") as psdp,
        ):
            # sample-0 load first in line for the DMA engines
            ghs = []
            gh0 = ghp.tile([P, 2 * Q, W], F16, tag="gh", name="gh_0")
            nc.gpsimd.dma_start(gh0[:], _in_full(0))
            ghs.append(gh0)

            pfsb = consts.tile([P, NS * NPF], F32)
            nc.sync.dma_start(pfsb[:], pf[:])
            pisb = consts.tile([1, NS], I32)
            nc.sync.dma_start(pisb[:], pi[:])
            ivsb = consts.tile([P, NS * Q], F16)
            nc.sync.dma_start(ivsb[:], iv[:])
            matsb = consts.tile([P, NS * 3 * P], F16)
            nc.sync.dma_start(matsb[:], mats[:])
            osb = consts.tile([1, NS * W], F32)

            # zero the scratch tails (window overreach past row 511)
            zp = consts.tile([P, RPAD * W // P], F8)
            nc.vector.memset(zp[:], 0.0)
            for s in range(NS):
                nc.sync.dma_start(
                    bass.AP(tensor=gds[s], offset=H * W,
                            ap=[[RPAD * W // P, P], [1, RPAD * W // P]]),
                    zp[:])

            # remaining input loads (Pool queue)
            for s in range(1, NS):
                t = ghp.tile([P, 2 * Q, W], F16, tag="gh", name=f"gh_{s}")
                nc.gpsimd.dma_start(t[:], _in_full(s))
                ghs.append(t)

            state = {}

            def emit_front(s):
                """combine -> fp8 scratch write -> window read for sample s."""
                pcol = lambda c: pfsb[:, s * NPF + c: s * NPF + c + 1]
                gh = ghs[s]
                gsb = work.tile([P, Q, W], F16, tag="G", name=f"G_{s}")
                for half, lo in ((0, 0), (1, 2)):
                    t0 = work.tile([P, 2, W], F16, tag=f"t0{half}",
                                   name=f"t0{half}_{s}")
                    nc.vector.tensor_scalar(
                        out=t0[:], in0=gh[:, lo:lo + 2, :], scalar1=pcol(COL_A),
                        scalar2=None, op0=mybir.AluOpType.mult)
                    t1 = work.tile([P, 2, W], F16, tag=f"t1{half}",
                                   name=f"t1{half}_{s}")
                    nc.vector.tensor_scalar(
                        out=t1[:], in0=gh[:, Q + lo:Q + lo + 2, :],
                        scalar1=pcol(COL_B),
                        scalar2=None, op0=mybir.AluOpType.mult)
                    nc.vector.tensor_tensor(out=gsb[:, lo:lo + 2, :],
                                            in0=t0[:], in1=t1[:],
                                            op=mybir.AluOpType.add)

                # fp8 cast write (Pool SWDGE); the SP-side window read is
                # ordered behind it by a post-build pass that makes it wait
                # on the write's own queue-completion semaphore
                wr = nc.gpsimd.dma_start(
                    bass.AP(tensor=gds[s], offset=0,
                            ap=[[W, P], [P * W, Q], [1, W]]),
                    gsb[:])
                off = nc.values_load(pisb[0:1, s: s + 1],
                                     engines=(mybir.EngineType.SP,),
                                     skip_runtime_bounds_check=True)
                win = winp.tile([P, Q, WLEN], F8, tag="win", name=f"win_{s}")
                rd = nc.sync.dma_start(
                    win[:], bass.AP(tensor=gds[s], offset=off,
                                    ap=[[W, P], [P * W, Q], [1, WLEN]]))
                wr_rd_pairs.append((wr.ins.name, rd.ins.name))

                psd = psdp.tile([P, Q, W], F32)
                sq = work.tile([P, Q, W], F16, tag="sq", name=f"sq_{s}")
                state[s] = dict(gsb=gsb, win=win, psd=psd, sq=sq)

            def emit_mms(s, q):
                st = state[s]
                mA = matsb[:, (3 * s) * P:(3 * s + 1) * P]
                mB0 = matsb[:, (3 * s + 1) * P:(3 * s + 2) * P]
                mB1 = matsb[:, (3 * s + 2) * P:(3 * s + 3) * P]
                psd, gsb, win = st["psd"], st["gsb"], st["win"]
                nc.tensor.matmul(psd[:, q, :], lhsT=mA, rhs=gsb[:, q, :],
                                 start=True, stop=False)
                nc.tensor.matmul(psd[:, q, :], lhsT=mB0, rhs=win[:, q, 0:W],
                                 start=False, stop=False)
                nc.tensor.matmul(psd[:, q, :], lhsT=mB1, rhs=win[:, q, 1:WLEN],
                                 start=False, stop=True)

            def emit_sq(s, q):
                st = state[s]
                nc.scalar.activation(st["sq"][:, q, :], st["psd"][:, q, :],
                                     mybir.ActivationFunctionType.Square)

            def emit_cs(s, q):
                st = state[s]
                ps = st["psd"][0:1, 0, 0:W]
                nc.tensor.matmul(ps, lhsT=ivsb[:, s * Q + q: s * Q + q + 1],
                                 rhs=st["sq"][:, q, :],
                                 start=(q == 0), stop=(q == Q - 1))
                if q == Q - 1:
                    nc.vector.tensor_copy(osb[0:1, s * W:(s + 1) * W], ps)
                    nc.sync.dma_start(out[s: s + 1, :],
                                      osb[0:1, s * W:(s + 1) * W])

            # fine-grained software pipeline over (s, q) blocks
            from collections import deque
            pending = deque()
            for s in range(NS):
                emit_front(s)
                for q in range(Q):
                    emit_mms(s, q)
                    if len(pending) >= 2:
                        emit_cs(*pending.popleft())
                    emit_sq(s, q)
                    pending.append((s, q))
            while pending:
                emit_cs(*pending.popleft())

    _order_scratch_reads(nc, wr_rd_pairs)
    return nc


def _order_scratch_reads(nc, wr_rd_pairs):
    """Make each SP window read wait on its Pool scratch write's completion
    (cross-queue DRAM deps aren't auto-tracked). Each DMA updates one of the
    rotating queue sems; the wait value is the cumulative update count of
    that sem up to and including the write."""
    wr_names = {w: r for (w, r) in wr_rd_pairs}
    cum = {}
    waits_for_rd = {}
    for fn in nc.m.functions:
        for bb in fn.blocks:
            for ins in bb.instructions:
                si = ins.sync_info
                if si is None or not si.on_update:
                    continue
                for u in si.on_update:
                    if getattr(u, "sync_type", None) != "semaphore":
                        continue
                    key = u.id
                    cum[key] = cum.get(key, 0) + (u.update_value or 0)
                    if ins.name in wr_names and ins.opcode == "DMACopy":
                        waits_for_rd[wr_names[ins.name]] = (
                            u.id, u.ant_name, cum[key])
    fixed = 0
    for fn in nc.m.functions:
        for bb in fn.blocks:
            for ins in bb.instructions:
                if ins.name in waits_for_rd:
                    sid, sname, val = waits_for_rd[ins.name]
                    si = ins.sync_info or mybir.SyncInfo(on_wait=[],
                                                         on_update=[])
                    si.on_wait = list(si.on_wait or []) + [mybir.SyncWait(
                        sync_type="semaphore", id=sid, ant_name=sname,
                        wait_mode="sem-ge-imm", wait_value=val)]
                    ins.sync_info = si
                    fixed += 1
    assert fixed == len(wr_rd_pairs), (fixed, len(wr_rd_pairs))


def _host_params(gt_sym_axis, gd_sym_axis):
    B = gt_sym_axis.shape[0]
    gt = gt_sym_axis.astype(np.float32)
    gds = gd_sym_axis.astype(np.float32)
    prm = []
    for i in range(B):
        sx = gds[i, 0]
        sy = gds[i, 1]
        dx = np.float32(-10.0) * gt[i, 0]
        dy = np.float32(10.0) * gt[i, 1]
        dy1f = np.float32(np.floor(dy))
        dx1f = np.float32(np.floor(dx))
        dy1 = int(dy1f)
        dx1 = int(dx1f)
        fy = np.float32(dy - dy1f)
        fx = np.float32(dx - dx1f)
        pos = bool(dx > 0)
        one = np.float32(1.0)
        zero = np.float32(0.0)
        if pos:
            wa = (one, zero)
            wb = (-(one - fy), -fy)
            rb, cb = dy1, dx1
            jlo, jhi = 0, W - dx1 - 1
        else:
            wa = (-fy, -(one - fy))
            wb = (one, zero)
            rb, cb = dy1 + 1, dx1
            jlo, jhi = -dx1, W
        rows = H - dy1 - 1
        cols = (W - dx1 - 1) if pos else (W + dx1)
        m = max(abs(float(sx)), abs(float(sy)), 1e-30)
        a = np.float32(float(sy) / m)
        b = np.float32(float(sx) / m)
        off = rb * W + cb
        assert 1 <= off and off + (H - 1) * W + WLEN <= R
        assert 0 <= rb <= RPAD - 2 and -16 <= cb <= 16
        assert 0 <= jlo <= jhi <= W
        prm.append(dict(a=a, b=b, wa=wa, wb=wb, rb=rb, cb=cb,
                        cb0=one - fx, cb1=fx, jlo=jlo, jhi=jhi,
                        rows=rows, cols=cols, scale=m * m))
    return prm


def _band(w0, w1):
    """lhsT[k, m] = w0*d(k==m) + w1*d(k==m+1), out-row 127 zeroed."""
    mat = np.zeros((P, P), np.float16)
    idx = np.arange(P)
    mat[idx, idx] = np.float16(w0)
    mat[idx[1:], idx[:-1]] = np.float16(w1)
    mat[:, P - 1] = np.float16(0.0)
    return mat


def _seam_fix(grid_s, p):
    """Exact fp64 contribution of the device-zeroed rows 127/255/383/511."""
    g0 = grid_s[0].astype(np.float64)
    g1 = grid_s[1].astype(np.float64)
    G = p["a"] * g0 + p["b"] * g1
    Gp = np.vstack([G, np.zeros((RPAD, W))])
    flat = Gp.reshape(-1)
    wa0, wa1 = float(p["wa"][0]), float(p["wa"][1])
    wb0, wb1 = float(p["wb"][0]), float(p["wb"][1])
    cb0, cb1 = float(p["cb0"]), float(p["cb1"])
    base = p["rb"] * W + p["cb"]
    jlo, jhi = p["jlo"], p["jhi"]
    ssq = 0.0
    for r in (127, 255, 383, 511):
        if r >= p["rows"]:
            continue
        w_r = flat[base + r * W: base + r * W + W + 1]
        w_r1 = flat[base + (r + 1) * W: base + (r + 1) * W + W + 1]
        bc_r = cb0 * w_r[0:W] + cb1 * w_r[1:W + 1]
        bc_r1 = cb0 * w_r1[0:W] + cb1 * w_r1[1:W + 1]
        g_r1 = G[r + 1] if r + 1 < H else np.zeros(W)
        d = wa0 * G[r] + wa1 * g_r1 + wb0 * bc_r + wb1 * bc_r1
        ssq += float((d[jlo:jhi] ** 2).sum())
    return ssq


def kernel(grid, gt_sym_axis, gd_sym_axis):
    grid = np.ascontiguousarray(grid, dtype=np.float32)
    B = grid.shape[0]
    assert grid.shape == (B, 2, H, W) and B == NS * NCORES

    if "nc" not in _CACHE:
        nc = _build_program()
        _split_multiwaits(nc)
        _CACHE["nc"] = nc
    nc = _CACHE["nc"]

    prm = _host_params(np.asarray(gt_sym_axis), np.asarray(gd_sym_axis))

    i_of_pq = np.arange(H).reshape(Q, P).T
    in_maps = []
    for c in range(NCORES):
        pfv = np.zeros((P, NS * NPF), np.float32)
        piv = np.zeros((1, NS), np.int32)
        ivv = np.zeros((P, NS * Q), np.float16)
        matv = np.zeros((P, NS * 3 * P), np.float16)
        for s in range(NS):
            p = prm[c * NS + s]
            pfv[:, s * NPF + COL_A] = p["a"]
            pfv[:, s * NPF + COL_B] = p["b"]
            piv[0, s] = p["rb"] * W + p["cb"]
            ivv[:, s * Q:(s + 1) * Q] = (i_of_pq < p["rows"]).astype(np.float16)
            matv[:, (3 * s) * P:(3 * s + 1) * P] = _band(*p["wa"])
            bb = _band(*p["wb"])
            matv[:, (3 * s + 1) * P:(3 * s + 2) * P] = (
                bb * np.float16(p["cb0"]))
            matv[:, (3 * s + 2) * P:(3 * s + 3) * P] = (
                bb * np.float16(p["cb1"]))
        in_maps.append({
            "g": grid[c * NS:(c + 1) * NS],
            "pf": pfv, "pi": piv, "iv": ivv, "mats": matv,
        })

    res = run_bass_kernel_spmd(nc, in_maps, core_ids=list(range(NCORES)))

    losses = np.zeros(B, np.float64)
    for c in range(NCORES):
        o = res.results[c]["out"]
        for s in range(NS):
            p = prm[c * NS + s]
            ssq = float(o[s, p["jlo"]:p["jhi"]].sum(dtype=np.float64))
            ssq += _seam_fix(grid[c * NS + s], p)
            count = float(np.float32(p["rows"] * p["cols"]))
            losses[c * NS + s] = p["scale"] * ssq / count
    return np.float32(losses.mean())


# revision 25
# speedup vs baseline: 1.4098x; 1.1062x over previous
"""Trainium2 Bass kernel for nn_BilinearFullSymLoss (v5).

Per-sample math (validated against reference in fp64):
  delta(i,j) = wa0*G(i,j) + wa1*G(i+1,j) + wb0*bc(i,j) + wb1*bc(i+1,j)
  bc(i,j)    = cb0*Wn(i,j) + cb1*Wn(i,j+1),  Wn(i,j) = G(i+rb, j+cb)
  pos: wa=(1,0),         wb=(-(1-fy),-fy), rb=dy1,   cb=dx1, valid j in [0,W-dx1-1)
  neg: wa=(-fy,-(1-fy)), wb=(1,0),         rb=dy1+1, cb=dx1, valid j in [-dx1,W)
  loss = m^2 * sum(valid delta^2) / (rows*cols)

Device plan per core (4 samples):
- ONE Pool SWDGE casting DMA per sample loads both channels (f32->f16)
- DVE: G = a*g0 + b*g1 (ts, ts, tt per half; fp16 fast modes)
- G is cast-written (f16->fp8e4m3) by a Pool SWDGE DMA into a per-sample
  DRAM scratch whose 8 tail rows are zeroed; a manual semaphore orders the
  SP-side window read behind the Pool-side write (cross-queue DRAM deps
  are not tracked); ONE dynamic-offset window read
  win[p,q,0:W+1] = Gd[rb*W+cb + r*W + j] yields the row+column shift, fp8
- PE accumulates delta per 128-row block in PSUM with host-built banded
  lhsT (fp16) against mixed-precision rhs:
    psd[:,q,:] = mA@G[:,q,:] (fp16) + mB0@win[:,q,0:W] + mB1@win[:,q,1:W+1]
  (fp8 rhs); mA = wa0*I + wa1*sub, mB0/mB1 = cb0/cb1*(wb0*I + wb1*sub),
  all with out-row 127 zeroed: seam rows 127/255/383/511 are added back
  exactly on the host from the f32 grid, so no cross-seam matmuls exist
- ACT squares each 128-row block right after its three matmuls; the
  ivalid-weighted column-sum matmul for block q trails by one block so PE
  never waits on ACT
- DVE copies [1,W] per sample to SBUF; per-sample DMA writes the output.
Host: sums the valid column range, adds seam rows, scales, means.
"""

import sys

sys.path.insert(0, "/opt/trn_rl_repo")

import numpy as np

import concourse.bass as bass
import concourse.tile as tile
from concourse import mybir
from concourse.bass_utils import run_bass_kernel_spmd

H = 512
W = 512
P = 128
Q = H // P
NS = 4
NCORES = 8
WLEN = W + 1
RPAD = 8
R = (H + RPAD) * W

F32 = mybir.dt.float32
F16 = mybir.dt.float16
F8 = mybir.dt.float8e4
I32 = mybir.dt.int32

NPF = 2  # a, b
COL_A, COL_B = range(NPF)

_CACHE = {}


def _split_multiwaits(nc):
    """The staged walrus accepts one sync wait per instruction; hoist extras
    onto single-wait NoOps."""
    n = 0
    for fn in nc.m.functions:
        for bb in fn.blocks:
            newlist = []
            for ins in bb.instructions:
                si = ins.sync_info
                if si is not None and si.on_wait is not None and len(si.on_wait) > 1:
                    waits = list(si.on_wait)
                    for w in waits[:-1]:
                        n += 1
                        newlist.append(mybir.InstNoOp(
                            name=f"WSPLIT-{n}-{ins.name}", opcode="NoOp",
                            engine=ins.engine,
                            sync_info=mybir.SyncInfo(on_wait=[w], on_update=[])))
                    ins.sync_info = mybir.SyncInfo(
                        on_wait=[waits[-1]], on_update=list(si.on_update))
                newlist.append(ins)
            bb.instructions = newlist
    return n


def _build_program():
    nc = bass.Bass("TRN2", target_bir_lowering=False, debug=False)

    g = nc.dram_tensor("g", [NS, 2, H, W], F32, kind="ExternalInput")
    pf = nc.dram_tensor("pf", [P, NS * NPF], F32, kind="ExternalInput")
    pi = nc.dram_tensor("pi", [1, NS], I32, kind="ExternalInput")
    iv = nc.dram_tensor("iv", [P, NS * Q], F16, kind="ExternalInput")
    mats = nc.dram_tensor("mats", [P, NS * 3 * P], F16, kind="ExternalInput")
    out = nc.dram_tensor("out", [NS, W], F32, kind="ExternalOutput")
    gds = [nc.dram_tensor(f"gd{s}", [R, 1], F8) for s in range(NS)]

    def _in_full(s):
        # both channels, all rows: (c, q) merges into one contiguous dim
        return bass.AP(tensor=g, offset=s * 2 * H * W,
                       ap=[[W, P], [P * W, 2 * Q], [1, W]])

    wr_rd_pairs = []

    with tile.TileContext(nc) as tc:
        with (
            tc.tile_pool(name="consts", bufs=1) as consts,
            tc.tile_pool(name="gh", bufs=4) as ghp,
            tc.tile_pool(name="win", bufs=3) as winp,
            tc.tile_pool(name="work", bufs=3) as work,
            tc.tile_pool(name="sq", bufs=4) as sqp,
            tc.tile_pool(name="psd", bufs=2, space="PSUM") as psdp,
            tc.tile_pool(name="csp", bufs=1, space="PSUM") as csp,
        ):
            # sample-0 load first in line for the DMA engines
            ghs = []
            gh0 = ghp.tile([P, 2 * Q, W], F16, tag="gh", name="gh_0")
            nc.gpsimd.dma_start(gh0[:], _in_full(0))
            ghs.append(gh0)

            pfsb = consts.tile([P, NS * NPF], F32)
            nc.sync.dma_start(pfsb[:], pf[:])
            pisb = consts.tile([1, NS], I32)
            nc.sync.dma_start(pisb[:], pi[:])
            ivsb = consts.tile([P, NS * Q], F16)
            nc.sync.dma_start(ivsb[:], iv[:])
            matsb = consts.tile([P, NS * 3 * P], F16)
            nc.sync.dma_start(matsb[:], mats[:])
            osb = consts.tile([1, NS * W], F32)

            # zero the scratch tails (window overreach past row 511)
            zp = consts.tile([P, RPAD * W // P], F8)
            nc.vector.memset(zp[:], 0.0)
            for s in range(NS):
                nc.sync.dma_start(
                    bass.AP(tensor=gds[s], offset=H * W,
                            ap=[[RPAD * W // P, P], [1, RPAD * W // P]]),
                    zp[:])

            # remaining input loads (Pool queue)
            for s in range(1, NS):
                t = ghp.tile([P, 2 * Q, W], F16, tag="gh", name=f"gh_{s}")
                nc.gpsimd.dma_start(t[:], _in_full(s))
                ghs.append(t)

            state = {}

            def emit_front(s):
                """combine -> fp8 scratch write -> window read for sample s."""
                pcol = lambda c: pfsb[:, s * NPF + c: s * NPF + c + 1]
                gh = ghs[s]
                gsb = work.tile([P, Q, W], F16, tag="G", name=f"G_{s}")
                for half, lo in ((0, 0), (1, 2)):
                    t0 = work.tile([P, 2, W], F16, tag=f"t0{half}",
                                   name=f"t0{half}_{s}")
                    nc.vector.tensor_scalar(
                        out=t0[:], in0=gh[:, lo:lo + 2, :], scalar1=pcol(COL_A),
                        scalar2=None, op0=mybir.AluOpType.mult)
                    t1 = work.tile([P, 2, W], F16, tag=f"t1{half}",
                                   name=f"t1{half}_{s}")
                    nc.vector.tensor_scalar(
                        out=t1[:], in0=gh[:, Q + lo:Q + lo + 2, :],
                        scalar1=pcol(COL_B),
                        scalar2=None, op0=mybir.AluOpType.mult)
                    nc.vector.tensor_tensor(out=gsb[:, lo:lo + 2, :],
                                            in0=t0[:], in1=t1[:],
                                            op=mybir.AluOpType.add)

                # fp8 cast write (Pool SWDGE); the SP-side window read is
                # ordered behind it by a post-build pass that makes it wait
                # on the write's own queue-completion semaphore
                wr = nc.gpsimd.dma_start(
                    bass.AP(tensor=gds[s], offset=0,
                            ap=[[W, P], [P * W, Q], [1, W]]),
                    gsb[:])
                off = nc.values_load(pisb[0:1, s: s + 1],
                                     engines=(mybir.EngineType.SP,),
                                     skip_runtime_bounds_check=True)
                win = winp.tile([P, Q, WLEN], F8, tag="win", name=f"win_{s}")
                rd = nc.sync.dma_start(
                    win[:], bass.AP(tensor=gds[s], offset=off,
                                    ap=[[W, P], [P * W, Q], [1, WLEN]]))
                wr_rd_pairs.append((wr.ins.name, rd.ins.name))

                sq = sqp.tile([P, Q, W], F16, tag="sq", name=f"sq_{s}")
                state[s] = dict(gsb=gsb, win=win, sq=sq)

            def emit_delta(s, half):
                """two 128-row blocks -> half-sample PSUM tile -> squares."""
                st = state[s]
                mA = matsb[:, (3 * s) * P:(3 * s + 1) * P]
                mB0 = matsb[:, (3 * s + 1) * P:(3 * s + 2) * P]
                mB1 = matsb[:, (3 * s + 2) * P:(3 * s + 3) * P]
                gsb, win = st["gsb"], st["win"]
                psdh = psdp.tile([P, 2, W], F32)
                for qq in range(2):
                    q = 2 * half + qq
                    nc.tensor.matmul(psdh[:, qq, :], lhsT=mA, rhs=gsb[:, q, :],
                                     start=True, stop=False)
                    nc.tensor.matmul(psdh[:, qq, :], lhsT=mB0,
                                     rhs=win[:, q, 0:W],
                                     start=False, stop=False)
                    nc.tensor.matmul(psdh[:, qq, :], lhsT=mB1,
                                     rhs=win[:, q, 1:WLEN],
                                     start=False, stop=True)
                    nc.scalar.activation(st["sq"][:, q, :], psdh[:, qq, :],
                                         mybir.ActivationFunctionType.Square)

            for s in range(NS):
                emit_front(s)
                for half in (0, 1):
                    emit_delta(s, half)

            # column sums trail; the scheduler slots them into PE gaps.
            # one single-bank accumulator per sample, all at partition 0
            # (the walrus verifier rejects cross-partition-base copies)
            csts = [csp.tile([P, W], F32, name=f"cst{s}") for s in range(NS)]
            for s in range(NS):
                for q in range(Q):
                    nc.tensor.matmul(
                        csts[s][0:1, :],
                        lhsT=ivsb[:, s * Q + q: s * Q + q + 1],
                        rhs=state[s]["sq"][:, q, :],
                        start=(q == 0), stop=(q == Q - 1))
                nc.vector.tensor_copy(osb[0:1, s * W:(s + 1) * W],
                                      csts[s][0:1, :])
            nc.sync.dma_start(out[:], osb[0:1, :])

    _order_scratch_reads(nc, wr_rd_pairs)
    return nc


def _order_scratch_reads(nc, wr_rd_pairs):
    """Make each SP window read wait on its Pool scratch write's completion
    (cross-queue DRAM deps aren't auto-tracked). Each DMA updates one of the
    rotating queue sems; the wait value is the cumulative update count of
    that sem up to and including the write."""
    wr_names = {w: r for (w, r) in wr_rd_pairs}
    cum = {}
    waits_for_rd = {}
    for fn in nc.m.functions:
        for bb in fn.blocks:
            for ins in bb.instructions:
                si = ins.sync_info
                if si is None or not si.on_update:
                    continue
                for u in si.on_update:
                    if getattr(u, "sync_type", None) != "semaphore":
                        continue
                    key = u.id
                    cum[key] = cum.get(key, 0) + (u.update_value or 0)
                    if ins.name in wr_names and ins.opcode == "DMACopy":
                        waits_for_rd[wr_names[ins.name]] = (
                            u.id, u.ant_name, cum[key])
    fixed = 0
    for fn in nc.m.functions:
        for bb in fn.blocks:
            for ins in bb.instructions:
                if ins.name in waits_for_rd:
                    sid, sname, val = waits_for_rd[ins.name]
                    si = ins.sync_info or mybir.SyncInfo(on_wait=[],
                                                         on_update=[])
                    si.on_wait = list(si.on_wait or []) + [mybir.SyncWait(
                        sync_type="semaphore", id=sid, ant_name=sname,
                        wait_mode="sem-ge-imm", wait_value=val)]
                    ins.sync_info = si
                    fixed += 1
    assert fixed == len(wr_rd_pairs), (fixed, len(wr_rd_pairs))


def _host_params(gt_sym_axis, gd_sym_axis):
    B = gt_sym_axis.shape[0]
    gt = gt_sym_axis.astype(np.float32)
    gds = gd_sym_axis.astype(np.float32)
    prm = []
    for i in range(B):
        sx = gds[i, 0]
        sy = gds[i, 1]
        dx = np.float32(-10.0) * gt[i, 0]
        dy = np.float32(10.0) * gt[i, 1]
        dy1f = np.float32(np.floor(dy))
        dx1f = np.float32(np.floor(dx))
        dy1 = int(dy1f)
        dx1 = int(dx1f)
        fy = np.float32(dy - dy1f)
        fx = np.float32(dx - dx1f)
        pos = bool(dx > 0)
        one = np.float32(1.0)
        zero = np.float32(0.0)
        if pos:
            wa = (one, zero)
            wb = (-(one - fy), -fy)
            rb, cb = dy1, dx1
            jlo, jhi = 0, W - dx1 - 1
        else:
            wa = (-fy, -(one - fy))
            wb = (one, zero)
            rb, cb = dy1 + 1, dx1
            jlo, jhi = -dx1, W
        rows = H - dy1 - 1
        cols = (W - dx1 - 1) if pos else (W + dx1)
        m = max(abs(float(sx)), abs(float(sy)), 1e-30)
        a = np.float32(float(sy) / m)
        b = np.float32(float(sx) / m)
        off = rb * W + cb
        assert 1 <= off and off + (H - 1) * W + WLEN <= R
        assert 0 <= rb <= RPAD - 2 and -16 <= cb <= 16
        assert 0 <= jlo <= jhi <= W
        prm.append(dict(a=a, b=b, wa=wa, wb=wb, rb=rb, cb=cb,
                        cb0=one - fx, cb1=fx, jlo=jlo, jhi=jhi,
                        rows=rows, cols=cols, scale=m * m))
    return prm


def _band(w0, w1):
    """lhsT[k, m] = w0*d(k==m) + w1*d(k==m+1), out-row 127 zeroed."""
    mat = np.zeros((P, P), np.float16)
    idx = np.arange(P)
    mat[idx, idx] = np.float16(w0)
    mat[idx[1:], idx[:-1]] = np.float16(w1)
    mat[:, P - 1] = np.float16(0.0)
    return mat


def _seam_fix(grid_s, p):
    """Exact fp64 contribution of the device-zeroed rows 127/255/383/511."""
    g0 = grid_s[0].astype(np.float64)
    g1 = grid_s[1].astype(np.float64)
    G = p["a"] * g0 + p["b"] * g1
    Gp = np.vstack([G, np.zeros((RPAD, W))])
    flat = Gp.reshape(-1)
    wa0, wa1 = float(p["wa"][0]), float(p["wa"][1])
    wb0, wb1 = float(p["wb"][0]), float(p["wb"][1])
    cb0, cb1 = float(p["cb0"]), float(p["cb1"])
    base = p["rb"] * W + p["cb"]
    jlo, jhi = p["jlo"], p["jhi"]
    ssq = 0.0
    for r in (127, 255, 383, 511):
        if r >= p["rows"]:
            continue
        w_r = flat[base + r * W: base + r * W + W + 1]
        w_r1 = flat[base + (r + 1) * W: base + (r + 1) * W + W + 1]
        bc_r = cb0 * w_r[0:W] + cb1 * w_r[1:W + 1]
        bc_r1 = cb0 * w_r1[0:W] + cb1 * w_r1[1:W + 1]
        g_r1 = G[r + 1] if r + 1 < H else np.zeros(W)
        d = wa0 * G[r] + wa1 * g_r1 + wb0 * bc_r + wb1 * bc_r1
        ssq += float((d[jlo:jhi] ** 2).sum())
    return ssq


def kernel(grid, gt_sym_axis, gd_sym_axis):
    grid = np.ascontiguousarray(grid, dtype=np.float32)
    B = grid.shape[0]
    assert grid.shape == (B, 2, H, W) and B == NS * NCORES

    if "nc" not in _CACHE:
        nc = _build_program()
        _split_multiwaits(nc)
        _CACHE["nc"] = nc
    nc = _CACHE["nc"]

    prm = _host_params(np.asarray(gt_sym_axis), np.asarray(gd_sym_axis))

    i_of_pq = np.arange(H).reshape(Q, P).T
    in_maps = []
    for c in range(NCORES):
        pfv = np.zeros((P, NS * NPF), np.float32)
        piv = np.zeros((1, NS), np.int32)
        ivv = np.zeros((P, NS * Q), np.float16)
        matv = np.zeros((P, NS * 3 * P), np.float16)
        for s in range(NS):
            p = prm[c * NS + s]
            pfv[:, s * NPF + COL_A] = p["a"]
            pfv[:, s * NPF + COL_B] = p["b"]
            piv[0, s] = p["rb"] * W + p["cb"]
            ivv[:, s * Q:(s + 1) * Q] = (i_of_pq < p["rows"]).astype(np.float16)
            matv[:, (3 * s) * P:(3 * s + 1) * P] = _band(*p["wa"])
            bb = _band(*p["wb"])
            matv[:, (3 * s + 1) * P:(3 * s + 2) * P] = (
                bb * np.float16(p["cb0"]))
            matv[:, (3 * s + 2) * P:(3 * s + 3) * P] = (
                bb * np.float16(p["cb1"]))
        in_maps.append({
            "g": grid[c * NS:(c + 1) * NS],
            "pf": pfv, "pi": piv, "iv": ivv, "mats": matv,
        })

    res = run_bass_kernel_spmd(nc, in_maps, core_ids=list(range(NCORES)))

    losses = np.zeros(B, np.float64)
    for c in range(NCORES):
        o = res.results[c]["out"]
        for s in range(NS):
            p = prm[c * NS + s]
            ssq = float(o[s, p["jlo"]:p["jhi"]].sum(dtype=np.float64))
            ssq += _seam_fix(grid[c * NS + s], p)
            count = float(np.float32(p["rows"] * p["cols"]))
            losses[c * NS + s] = p["scale"] * ssq / count
    return np.float32(losses.mean())


# revision 26
# speedup vs baseline: 1.4196x; 1.0069x over previous
"""Trainium2 Bass kernel for nn_BilinearFullSymLoss (v5).

Per-sample math (validated against reference in fp64):
  delta(i,j) = wa0*G(i,j) + wa1*G(i+1,j) + wb0*bc(i,j) + wb1*bc(i+1,j)
  bc(i,j)    = cb0*Wn(i,j) + cb1*Wn(i,j+1),  Wn(i,j) = G(i+rb, j+cb)
  pos: wa=(1,0),         wb=(-(1-fy),-fy), rb=dy1,   cb=dx1, valid j in [0,W-dx1-1)
  neg: wa=(-fy,-(1-fy)), wb=(1,0),         rb=dy1+1, cb=dx1, valid j in [-dx1,W)
  loss = m^2 * sum(valid delta^2) / (rows*cols)

Device plan per core (4 samples):
- ONE Pool SWDGE casting DMA per sample loads both channels (f32->f16)
- DVE: G = a*g0 + b*g1 (ts, ts, tt per half; fp16 fast modes)
- G is cast-written (f16->fp8e4m3) by a Pool SWDGE DMA into a per-sample
  DRAM scratch whose 8 tail rows are zeroed; a manual semaphore orders the
  SP-side window read behind the Pool-side write (cross-queue DRAM deps
  are not tracked); ONE dynamic-offset window read
  win[p,q,0:W+1] = Gd[rb*W+cb + r*W + j] yields the row+column shift, fp8
- PE accumulates delta per 128-row block in PSUM with host-built banded
  lhsT (fp16) against mixed-precision rhs:
    psd[:,q,:] = mA@G[:,q,:] (fp16) + mB0@win[:,q,0:W] + mB1@win[:,q,1:W+1]
  (fp8 rhs); mA = wa0*I + wa1*sub, mB0/mB1 = cb0/cb1*(wb0*I + wb1*sub),
  all with out-row 127 zeroed: seam rows 127/255/383/511 are added back
  exactly on the host from the f32 grid, so no cross-seam matmuls exist
- ACT squares each 128-row block right after its three matmuls; the
  ivalid-weighted column-sum matmul for block q trails by one block so PE
  never waits on ACT
- DVE copies [1,W] per sample to SBUF; per-sample DMA writes the output.
Host: sums the valid column range, adds seam rows, scales, means.
"""

import sys

sys.path.insert(0, "/opt/trn_rl_repo")

import numpy as np

import concourse.bass as bass
import concourse.tile as tile
from concourse import mybir
from concourse.bass_utils import run_bass_kernel_spmd

H = 512
W = 512
P = 128
Q = H // P
NS = 4
NCORES = 8
WLEN = W + 1
RPAD = 8
R = (H + RPAD) * W

F32 = mybir.dt.float32
F16 = mybir.dt.float16
F8 = mybir.dt.float8e4
I32 = mybir.dt.int32

NPF = 2  # a, b
COL_A, COL_B = range(NPF)

_CACHE = {}


def _split_multiwaits(nc):
    """The staged walrus accepts one sync wait per instruction; hoist extras
    onto single-wait NoOps."""
    n = 0
    for fn in nc.m.functions:
        for bb in fn.blocks:
            newlist = []
            for ins in bb.instructions:
                si = ins.sync_info
                if si is not None and si.on_wait is not None and len(si.on_wait) > 1:
                    waits = list(si.on_wait)
                    for w in waits[:-1]:
                        n += 1
                        newlist.append(mybir.InstNoOp(
                            name=f"WSPLIT-{n}-{ins.name}", opcode="NoOp",
                            engine=ins.engine,
                            sync_info=mybir.SyncInfo(on_wait=[w], on_update=[])))
                    ins.sync_info = mybir.SyncInfo(
                        on_wait=[waits[-1]], on_update=list(si.on_update))
                newlist.append(ins)
            bb.instructions = newlist
    return n


def _build_program():
    nc = bass.Bass("TRN2", target_bir_lowering=False, debug=False)

    g = nc.dram_tensor("g", [NS, 2, H, W], F32, kind="ExternalInput")
    pf = nc.dram_tensor("pf", [P, NS * NPF], F32, kind="ExternalInput")
    pi = nc.dram_tensor("pi", [1, NS], I32, kind="ExternalInput")
    iv = nc.dram_tensor("iv", [P, NS * Q], F16, kind="ExternalInput")
    mats = nc.dram_tensor("mats", [P, NS * 3 * P], F16, kind="ExternalInput")
    out = nc.dram_tensor("out", [NS, W], F32, kind="ExternalOutput")
    gds = [nc.dram_tensor(f"gd{s}", [R, 1], F8) for s in range(NS)]

    def _in_full(s):
        # both channels, all rows: (c, q) merges into one contiguous dim
        return bass.AP(tensor=g, offset=s * 2 * H * W,
                       ap=[[W, P], [P * W, 2 * Q], [1, W]])

    wr_rd_pairs = []

    with tile.TileContext(nc) as tc:
        with (
            tc.tile_pool(name="consts", bufs=1) as consts,
            tc.tile_pool(name="gh", bufs=3) as ghp,
            tc.tile_pool(name="win", bufs=4) as winp,
            tc.tile_pool(name="work", bufs=3) as work,
            tc.tile_pool(name="sq", bufs=4) as sqp,
            tc.tile_pool(name="gp", bufs=4) as gp,
            tc.tile_pool(name="psd", bufs=2, space="PSUM") as psdp,
            tc.tile_pool(name="csp", bufs=1, space="PSUM") as csp,
        ):
            # sample-0 load first in line for the DMA engines
            ghs = []
            gh0 = ghp.tile([P, 2 * Q, W], F16, tag="gh", name="gh_0")
            nc.gpsimd.dma_start(gh0[:], _in_full(0))
            ghs.append(gh0)

            pfsb = consts.tile([P, NS * NPF], F32)
            nc.sync.dma_start(pfsb[:], pf[:])
            pisb = consts.tile([1, NS], I32)
            nc.sync.dma_start(pisb[:], pi[:])
            ivsb = consts.tile([P, NS * Q], F16)
            nc.sync.dma_start(ivsb[:], iv[:])
            matsb = consts.tile([P, NS * 3 * P], F16)
            nc.sync.dma_start(matsb[:], mats[:])
            osb = consts.tile([1, NS * W], F32)

            # zero the scratch tails (window overreach past row 511)
            zp = consts.tile([P, RPAD * W // P], F8)
            nc.vector.memset(zp[:], 0.0)
            for s in range(NS):
                nc.sync.dma_start(
                    bass.AP(tensor=gds[s], offset=H * W,
                            ap=[[RPAD * W // P, P], [1, RPAD * W // P]]),
                    zp[:])

            # remaining input loads (Pool queue)
            for s in range(1, NS):
                t = ghp.tile([P, 2 * Q, W], F16, tag="gh", name=f"gh_{s}")
                nc.gpsimd.dma_start(t[:], _in_full(s))
                ghs.append(t)

            state = {}

            def emit_front(s):
                """combine -> fp8 scratch write -> window read for sample s."""
                pcol = lambda c: pfsb[:, s * NPF + c: s * NPF + c + 1]
                gh = ghs[s]
                gsb = gp.tile([P, Q, W], F16, tag="G", name=f"G_{s}")
                for half, lo in ((0, 0), (1, 2)):
                    t0 = work.tile([P, 2, W], F16, tag=f"t0{half}",
                                   name=f"t0{half}_{s}")
                    nc.vector.tensor_scalar(
                        out=t0[:], in0=gh[:, lo:lo + 2, :], scalar1=pcol(COL_A),
                        scalar2=None, op0=mybir.AluOpType.mult)
                    t1 = work.tile([P, 2, W], F16, tag=f"t1{half}",
                                   name=f"t1{half}_{s}")
                    nc.vector.tensor_scalar(
                        out=t1[:], in0=gh[:, Q + lo:Q + lo + 2, :],
                        scalar1=pcol(COL_B),
                        scalar2=None, op0=mybir.AluOpType.mult)
                    nc.vector.tensor_tensor(out=gsb[:, lo:lo + 2, :],
                                            in0=t0[:], in1=t1[:],
                                            op=mybir.AluOpType.add)

                # fp8 cast write (Pool SWDGE); the SP-side window read is
                # ordered behind it by a post-build pass that makes it wait
                # on the write's own queue-completion semaphore
                wr = nc.gpsimd.dma_start(
                    bass.AP(tensor=gds[s], offset=0,
                            ap=[[W, P], [P * W, Q], [1, W]]),
                    gsb[:])
                off = nc.values_load(pisb[0:1, s: s + 1],
                                     engines=(mybir.EngineType.SP,),
                                     skip_runtime_bounds_check=True)
                win = winp.tile([P, Q, WLEN], F8, tag="win", name=f"win_{s}")
                rd = nc.sync.dma_start(
                    win[:], bass.AP(tensor=gds[s], offset=off,
                                    ap=[[W, P], [P * W, Q], [1, WLEN]]))
                wr_rd_pairs.append((wr.ins.name, rd.ins.name))

                sq = sqp.tile([P, Q, W], F16, tag="sq", name=f"sq_{s}")
                state[s] = dict(gsb=gsb, win=win, sq=sq)

            def emit_delta(s, half):
                """two 128-row blocks -> half-sample PSUM tile -> squares."""
                st = state[s]
                mA = matsb[:, (3 * s) * P:(3 * s + 1) * P]
                mB0 = matsb[:, (3 * s + 1) * P:(3 * s + 2) * P]
                mB1 = matsb[:, (3 * s + 2) * P:(3 * s + 3) * P]
                gsb, win = st["gsb"], st["win"]
                psdh = psdp.tile([P, 2, W], F32)
                for qq in range(2):
                    q = 2 * half + qq
                    nc.tensor.matmul(psdh[:, qq, :], lhsT=mA, rhs=gsb[:, q, :],
                                     start=True, stop=False)
                    nc.tensor.matmul(psdh[:, qq, :], lhsT=mB0,
                                     rhs=win[:, q, 0:W],
                                     start=False, stop=False)
                    nc.tensor.matmul(psdh[:, qq, :], lhsT=mB1,
                                     rhs=win[:, q, 1:WLEN],
                                     start=False, stop=True)
                    nc.scalar.activation(st["sq"][:, q, :], psdh[:, qq, :],
                                         mybir.ActivationFunctionType.Square)

            for s in range(NS):
                emit_front(s)
                for half in (0, 1):
                    emit_delta(s, half)

            # column sums trail; the scheduler slots them into PE gaps.
            # one single-bank accumulator per sample, all at partition 0
            # (the walrus verifier rejects cross-partition-base copies)
            csts = [csp.tile([P, W], F32, name=f"cst{s}") for s in range(NS)]
            for s in range(NS):
                for q in range(Q):
                    nc.tensor.matmul(
                        csts[s][0:1, :],
                        lhsT=ivsb[:, s * Q + q: s * Q + q + 1],
                        rhs=state[s]["sq"][:, q, :],
                        start=(q == 0), stop=(q == Q - 1))
                nc.vector.tensor_copy(osb[0:1, s * W:(s + 1) * W],
                                      csts[s][0:1, :])
            nc.sync.dma_start(out[:], osb[0:1, :])

    _order_scratch_reads(nc, wr_rd_pairs)
    return nc


def _order_scratch_reads(nc, wr_rd_pairs):
    """Make each SP window read wait on its Pool scratch write's completion
    (cross-queue DRAM deps aren't auto-tracked). Each DMA updates one of the
    rotating queue sems; the wait value is the cumulative update count of
    that sem up to and including the write."""
    wr_names = {w: r for (w, r) in wr_rd_pairs}
    cum = {}
    waits_for_rd = {}
    for fn in nc.m.functions:
        for bb in fn.blocks:
            for ins in bb.instructions:
                si = ins.sync_info
                if si is None or not si.on_update:
                    continue
                for u in si.on_update:
                    if getattr(u, "sync_type", None) != "semaphore":
                        continue
                    key = u.id
                    cum[key] = cum.get(key, 0) + (u.update_value or 0)
                    if ins.name in wr_names and ins.opcode == "DMACopy":
                        waits_for_rd[wr_names[ins.name]] = (
                            u.id, u.ant_name, cum[key])
    fixed = 0
    for fn in nc.m.functions:
        for bb in fn.blocks:
            for ins in bb.instructions:
                if ins.name in waits_for_rd:
                    sid, sname, val = waits_for_rd[ins.name]
                    si = ins.sync_info or mybir.SyncInfo(on_wait=[],
                                                         on_update=[])
                    si.on_wait = list(si.on_wait or []) + [mybir.SyncWait(
                        sync_type="semaphore", id=sid, ant_name=sname,
                        wait_mode="sem-ge-imm", wait_value=val)]
                    ins.sync_info = si
                    fixed += 1
    assert fixed == len(wr_rd_pairs), (fixed, len(wr_rd_pairs))


def _host_params(gt_sym_axis, gd_sym_axis):
    B = gt_sym_axis.shape[0]
    gt = gt_sym_axis.astype(np.float32)
    gds = gd_sym_axis.astype(np.float32)
    prm = []
    for i in range(B):
        sx = gds[i, 0]
        sy = gds[i, 1]
        dx = np.float32(-10.0) * gt[i, 0]
        dy = np.float32(10.0) * gt[i, 1]
        dy1f = np.float32(np.floor(dy))
        dx1f = np.float32(np.floor(dx))
        dy1 = int(dy1f)
        dx1 = int(dx1f)
        fy = np.float32(dy - dy1f)
        fx = np.float32(dx - dx1f)
        pos = bool(dx > 0)
        one = np.float32(1.0)
        zero = np.float32(0.0)
        if pos:
            wa = (one, zero)
            wb = (-(one - fy), -fy)
            rb, cb = dy1, dx1
            jlo, jhi = 0, W - dx1 - 1
        else:
            wa = (-fy, -(one - fy))
            wb = (one, zero)
            rb, cb = dy1 + 1, dx1
            jlo, jhi = -dx1, W
        rows = H - dy1 - 1
        cols = (W - dx1 - 1) if pos else (W + dx1)
        m = max(abs(float(sx)), abs(float(sy)), 1e-30)
        a = np.float32(float(sy) / m)
        b = np.float32(float(sx) / m)
        off = rb * W + cb
        assert 1 <= off and off + (H - 1) * W + WLEN <= R
        assert 0 <= rb <= RPAD - 2 and -16 <= cb <= 16
        assert 0 <= jlo <= jhi <= W
        prm.append(dict(a=a, b=b, wa=wa, wb=wb, rb=rb, cb=cb,
                        cb0=one - fx, cb1=fx, jlo=jlo, jhi=jhi,
                        rows=rows, cols=cols, scale=m * m))
    return prm


def _band(w0, w1):
    """lhsT[k, m] = w0*d(k==m) + w1*d(k==m+1), out-row 127 zeroed."""
    mat = np.zeros((P, P), np.float16)
    idx = np.arange(P)
    mat[idx, idx] = np.float16(w0)
    mat[idx[1:], idx[:-1]] = np.float16(w1)
    mat[:, P - 1] = np.float16(0.0)
    return mat


def _seam_fix(grid_s, p):
    """Exact fp64 contribution of the device-zeroed rows 127/255/383/511."""
    g0 = grid_s[0].astype(np.float64)
    g1 = grid_s[1].astype(np.float64)
    G = p["a"] * g0 + p["b"] * g1
    Gp = np.vstack([G, np.zeros((RPAD, W))])
    flat = Gp.reshape(-1)
    wa0, wa1 = float(p["wa"][0]), float(p["wa"][1])
    wb0, wb1 = float(p["wb"][0]), float(p["wb"][1])
    cb0, cb1 = float(p["cb0"]), float(p["cb1"])
    base = p["rb"] * W + p["cb"]
    jlo, jhi = p["jlo"], p["jhi"]
    ssq = 0.0
    for r in (127, 255, 383, 511):
        if r >= p["rows"]:
            continue
        w_r = flat[base + r * W: base + r * W + W + 1]
        w_r1 = flat[base + (r + 1) * W: base + (r + 1) * W + W + 1]
        bc_r = cb0 * w_r[0:W] + cb1 * w_r[1:W + 1]
        bc_r1 = cb0 * w_r1[0:W] + cb1 * w_r1[1:W + 1]
        g_r1 = G[r + 1] if r + 1 < H else np.zeros(W)
        d = wa0 * G[r] + wa1 * g_r1 + wb0 * bc_r + wb1 * bc_r1
        ssq += float((d[jlo:jhi] ** 2).sum())
    return ssq


def kernel(grid, gt_sym_axis, gd_sym_axis):
    grid = np.ascontiguousarray(grid, dtype=np.float32)
    B = grid.shape[0]
    assert grid.shape == (B, 2, H, W) and B == NS * NCORES

    if "nc" not in _CACHE:
        nc = _build_program()
        _split_multiwaits(nc)
        _CACHE["nc"] = nc
    nc = _CACHE["nc"]

    prm = _host_params(np.asarray(gt_sym_axis), np.asarray(gd_sym_axis))

    i_of_pq = np.arange(H).reshape(Q, P).T
    in_maps = []
    for c in range(NCORES):
        pfv = np.zeros((P, NS * NPF), np.float32)
        piv = np.zeros((1, NS), np.int32)
        ivv = np.zeros((P, NS * Q), np.float16)
        matv = np.zeros((P, NS * 3 * P), np.float16)
        for s in range(NS):
            p = prm[c * NS + s]
            pfv[:, s * NPF + COL_A] = p["a"]
            pfv[:, s * NPF + COL_B] = p["b"]
            piv[0, s] = p["rb"] * W + p["cb"]
            ivv[:, s * Q:(s + 1) * Q] = (i_of_pq < p["rows"]).astype(np.float16)
            matv[:, (3 * s) * P:(3 * s + 1) * P] = _band(*p["wa"])
            bb = _band(*p["wb"])
            matv[:, (3 * s + 1) * P:(3 * s + 2) * P] = (
                bb * np.float16(p["cb0"]))
            matv[:, (3 * s + 2) * P:(3 * s + 3) * P] = (
                bb * np.float16(p["cb1"]))
        in_maps.append({
            "g": grid[c * NS:(c + 1) * NS],
            "pf": pfv, "pi": piv, "iv": ivv, "mats": matv,
        })

    res = run_bass_kernel_spmd(nc, in_maps, core_ids=list(range(NCORES)))

    losses = np.zeros(B, np.float64)
    for c in range(NCORES):
        o = res.results[c]["out"]
        for s in range(NS):
            p = prm[c * NS + s]
            ssq = float(o[s, p["jlo"]:p["jhi"]].sum(dtype=np.float64))
            ssq += _seam_fix(grid[c * NS + s], p)
            count = float(np.float32(p["rows"] * p["cols"]))
            losses[c * NS + s] = p["scale"] * ssq / count
    return np.float32(losses.mean())


# revision 27
# speedup vs baseline: 1.4792x; 1.0420x over previous
"""Trainium2 Bass kernel for nn_BilinearFullSymLoss (v5).

Per-sample math (validated against reference in fp64):
  delta(i,j) = wa0*G(i,j) + wa1*G(i+1,j) + wb0*bc(i,j) + wb1*bc(i+1,j)
  bc(i,j)    = cb0*Wn(i,j) + cb1*Wn(i,j+1),  Wn(i,j) = G(i+rb, j+cb)
  pos: wa=(1,0),         wb=(-(1-fy),-fy), rb=dy1,   cb=dx1, valid j in [0,W-dx1-1)
  neg: wa=(-fy,-(1-fy)), wb=(1,0),         rb=dy1+1, cb=dx1, valid j in [-dx1,W)
  loss = m^2 * sum(valid delta^2) / (rows*cols)

Device plan per core (4 samples):
- ONE Pool SWDGE casting DMA per sample loads both channels (f32->f16)
- DVE: G = a*g0 + b*g1 (ts, ts, tt per half; fp16 fast modes)
- G is cast-written (f16->fp8e4m3) by a Pool SWDGE DMA into a per-sample
  DRAM scratch whose 8 tail rows are zeroed; a manual semaphore orders the
  SP-side window read behind the Pool-side write (cross-queue DRAM deps
  are not tracked); ONE dynamic-offset window read
  win[p,q,0:W+1] = Gd[rb*W+cb + r*W + j] yields the row+column shift, fp8
- PE accumulates delta per 128-row block in PSUM with host-built banded
  lhsT (fp16) against mixed-precision rhs:
    psd[:,q,:] = mA@G[:,q,:] (fp16) + mB0@win[:,q,0:W] + mB1@win[:,q,1:W+1]
  (fp8 rhs); mA = wa0*I + wa1*sub, mB0/mB1 = cb0/cb1*(wb0*I + wb1*sub),
  all with out-row 127 zeroed: seam rows 127/255/383/511 are added back
  exactly on the host from the f32 grid, so no cross-seam matmuls exist
- ACT squares each 128-row block right after its three matmuls; the
  ivalid-weighted column-sum matmul for block q trails by one block so PE
  never waits on ACT
- DVE copies [1,W] per sample to SBUF; per-sample DMA writes the output.
Host: sums the valid column range, adds seam rows, scales, means.
"""

import sys

sys.path.insert(0, "/opt/trn_rl_repo")

import numpy as np

import concourse.bass as bass
import concourse.tile as tile
from concourse import mybir
from concourse.bass_utils import run_bass_kernel_spmd

H = 512
W = 512
P = 128
Q = H // P
NS = 4
NCORES = 8
WLEN = W + 1
RPAD = 8
R = (H + RPAD) * W

F32 = mybir.dt.float32
F16 = mybir.dt.float16
F8 = mybir.dt.float8e4
I32 = mybir.dt.int32

NPF = 2  # a, b
COL_A, COL_B = range(NPF)

_CACHE = {}


def _split_multiwaits(nc):
    """The staged walrus accepts one sync wait per instruction; hoist extras
    onto single-wait NoOps."""
    n = 0
    for fn in nc.m.functions:
        for bb in fn.blocks:
            newlist = []
            for ins in bb.instructions:
                si = ins.sync_info
                if si is not None and si.on_wait is not None and len(si.on_wait) > 1:
                    waits = list(si.on_wait)
                    for w in waits[:-1]:
                        n += 1
                        newlist.append(mybir.InstNoOp(
                            name=f"WSPLIT-{n}-{ins.name}", opcode="NoOp",
                            engine=ins.engine,
                            sync_info=mybir.SyncInfo(on_wait=[w], on_update=[])))
                    ins.sync_info = mybir.SyncInfo(
                        on_wait=[waits[-1]], on_update=list(si.on_update))
                newlist.append(ins)
            bb.instructions = newlist
    return n


def _build_program():
    nc = bass.Bass("TRN2", target_bir_lowering=False, debug=False)

    g = nc.dram_tensor("g", [NS, 2, H, W], F32, kind="ExternalInput")
    pf = nc.dram_tensor("pf", [P, NS * NPF], F32, kind="ExternalInput")
    pi = nc.dram_tensor("pi", [1, NS], I32, kind="ExternalInput")
    iv = nc.dram_tensor("iv", [P, NS * Q], F16, kind="ExternalInput")
    mats = nc.dram_tensor("mats", [P, NS * 3 * P], F16, kind="ExternalInput")
    out = nc.dram_tensor("out", [NS, W], F32, kind="ExternalOutput")
    gds = [nc.dram_tensor(f"gd{s}", [R, 1], F8) for s in range(NS)]

    def _in_full(s):
        # both channels, all rows: (c, q) merges into one contiguous dim
        return bass.AP(tensor=g, offset=s * 2 * H * W,
                       ap=[[W, P], [P * W, 2 * Q], [1, W]])

    wr_rd_pairs = []

    with tile.TileContext(nc) as tc:
        with (
            tc.tile_pool(name="consts", bufs=1) as consts,
            tc.tile_pool(name="gh", bufs=3) as ghp,
            tc.tile_pool(name="win", bufs=4) as winp,
            tc.tile_pool(name="work", bufs=3) as work,
            tc.tile_pool(name="sq", bufs=4) as sqp,
            tc.tile_pool(name="gp", bufs=4) as gp,
            tc.tile_pool(name="psd", bufs=2, space="PSUM") as psdp,
            tc.tile_pool(name="csp", bufs=1, space="PSUM") as csp,
        ):
            # sample-0 load first in line for the DMA engines
            ghs = []
            gh0 = ghp.tile([P, 2 * Q, W], F16, tag="gh", name="gh_0")
            nc.gpsimd.dma_start(gh0[:], _in_full(0))
            ghs.append(gh0)

            pfsb = consts.tile([P, NS * NPF], F32)
            nc.sync.dma_start(pfsb[:], pf[:])
            pisb = consts.tile([1, NS], I32)
            nc.sync.dma_start(pisb[:], pi[:])
            ivsb = consts.tile([P, NS * Q], F16)
            nc.sync.dma_start(ivsb[:], iv[:])
            matsb = consts.tile([P, NS * 3 * P], F16)
            nc.sync.dma_start(matsb[:], mats[:])
            osb = consts.tile([1, NS * W], F32)

            # zero the scratch tails (window overreach past row 511)
            zp = consts.tile([P, RPAD * W // P], F8)
            nc.vector.memset(zp[:], 0.0)
            for s in range(NS):
                nc.sync.dma_start(
                    bass.AP(tensor=gds[s], offset=H * W,
                            ap=[[RPAD * W // P, P], [1, RPAD * W // P]]),
                    zp[:])

            # sample-1 load follows immediately; loads 2 and 3 are emitted
            # after samples 0/1's scratch writes (Pool queue order) so the
            # first window read isn't starved by the input flood
            t = ghp.tile([P, 2 * Q, W], F16, tag="gh", name="gh_1")
            nc.gpsimd.dma_start(t[:], _in_full(1))
            ghs.append(t)

            state = {}

            def emit_load(s):
                t = ghp.tile([P, 2 * Q, W], F16, tag="gh", name=f"gh_{s}")
                nc.gpsimd.dma_start(t[:], _in_full(s))
                ghs.append(t)

            def emit_front(s):
                """combine -> fp8 scratch write -> window read for sample s."""
                pcol = lambda c: pfsb[:, s * NPF + c: s * NPF + c + 1]
                gh = ghs[s]
                gsb = gp.tile([P, Q, W], F16, tag="G", name=f"G_{s}")
                for half, lo in ((0, 0), (1, 2)):
                    t0 = work.tile([P, 2, W], F16, tag=f"t0{half}",
                                   name=f"t0{half}_{s}")
                    nc.vector.tensor_scalar(
                        out=t0[:], in0=gh[:, lo:lo + 2, :], scalar1=pcol(COL_A),
                        scalar2=None, op0=mybir.AluOpType.mult)
                    t1 = work.tile([P, 2, W], F16, tag=f"t1{half}",
                                   name=f"t1{half}_{s}")
                    nc.vector.tensor_scalar(
                        out=t1[:], in0=gh[:, Q + lo:Q + lo + 2, :],
                        scalar1=pcol(COL_B),
                        scalar2=None, op0=mybir.AluOpType.mult)
                    nc.vector.tensor_tensor(out=gsb[:, lo:lo + 2, :],
                                            in0=t0[:], in1=t1[:],
                                            op=mybir.AluOpType.add)

                # fp8 cast write (Pool SWDGE); the SP-side window read is
                # ordered behind it by a post-build pass that makes it wait
                # on the write's own queue-completion semaphore
                wr = nc.gpsimd.dma_start(
                    bass.AP(tensor=gds[s], offset=0,
                            ap=[[W, P], [P * W, Q], [1, W]]),
                    gsb[:])
                off = nc.values_load(pisb[0:1, s: s + 1],
                                     engines=(mybir.EngineType.SP,),
                                     skip_runtime_bounds_check=True)
                win = winp.tile([P, Q, WLEN], F8, tag="win", name=f"win_{s}")
                rd = nc.sync.dma_start(
                    win[:], bass.AP(tensor=gds[s], offset=off,
                                    ap=[[W, P], [P * W, Q], [1, WLEN]]))
                wr_rd_pairs.append((wr.ins.name, rd.ins.name))

                sq = sqp.tile([P, Q, W], F16, tag="sq", name=f"sq_{s}")
                state[s] = dict(gsb=gsb, win=win, sq=sq)

            def emit_delta(s, half):
                """two 128-row blocks -> half-sample PSUM tile -> squares."""
                st = state[s]
                mA = matsb[:, (3 * s) * P:(3 * s + 1) * P]
                mB0 = matsb[:, (3 * s + 1) * P:(3 * s + 2) * P]
                mB1 = matsb[:, (3 * s + 2) * P:(3 * s + 3) * P]
                gsb, win = st["gsb"], st["win"]
                psdh = psdp.tile([P, 2, W], F32)
                for qq in range(2):
                    q = 2 * half + qq
                    nc.tensor.matmul(psdh[:, qq, :], lhsT=mA, rhs=gsb[:, q, :],
                                     start=True, stop=False)
                    nc.tensor.matmul(psdh[:, qq, :], lhsT=mB0,
                                     rhs=win[:, q, 0:W],
                                     start=False, stop=False)
                    nc.tensor.matmul(psdh[:, qq, :], lhsT=mB1,
                                     rhs=win[:, q, 1:WLEN],
                                     start=False, stop=True)
                    nc.scalar.activation(st["sq"][:, q, :], psdh[:, qq, :],
                                         mybir.ActivationFunctionType.Square)

            for s in range(NS):
                emit_front(s)
                if s + 2 < NS:
                    emit_load(s + 2)
                for half in (0, 1):
                    emit_delta(s, half)

            # column sums trail; the scheduler slots them into PE gaps.
            # one single-bank accumulator per sample, all at partition 0
            # (the walrus verifier rejects cross-partition-base copies)
            csts = [csp.tile([P, W], F32, name=f"cst{s}") for s in range(NS)]
            for s in range(NS):
                for q in range(Q):
                    nc.tensor.matmul(
                        csts[s][0:1, :],
                        lhsT=ivsb[:, s * Q + q: s * Q + q + 1],
                        rhs=state[s]["sq"][:, q, :],
                        start=(q == 0), stop=(q == Q - 1))
                nc.vector.tensor_copy(osb[0:1, s * W:(s + 1) * W],
                                      csts[s][0:1, :])
            nc.sync.dma_start(out[:], osb[0:1, :])

    _order_scratch_reads(nc, wr_rd_pairs)
    return nc


def _order_scratch_reads(nc, wr_rd_pairs):
    """Make each SP window read wait on its Pool scratch write's completion
    (cross-queue DRAM deps aren't auto-tracked). Each DMA updates one of the
    rotating queue sems; the wait value is the cumulative update count of
    that sem up to and including the write."""
    wr_names = {w: r for (w, r) in wr_rd_pairs}
    cum = {}
    waits_for_rd = {}
    for fn in nc.m.functions:
        for bb in fn.blocks:
            for ins in bb.instructions:
                si = ins.sync_info
                if si is None or not si.on_update:
                    continue
                for u in si.on_update:
                    if getattr(u, "sync_type", None) != "semaphore":
                        continue
                    key = u.id
                    cum[key] = cum.get(key, 0) + (u.update_value or 0)
                    if ins.name in wr_names and ins.opcode == "DMACopy":
                        waits_for_rd[wr_names[ins.name]] = (
                            u.id, u.ant_name, cum[key])
    fixed = 0
    for fn in nc.m.functions:
        for bb in fn.blocks:
            for ins in bb.instructions:
                if ins.name in waits_for_rd:
                    sid, sname, val = waits_for_rd[ins.name]
                    si = ins.sync_info or mybir.SyncInfo(on_wait=[],
                                                         on_update=[])
                    si.on_wait = list(si.on_wait or []) + [mybir.SyncWait(
                        sync_type="semaphore", id=sid, ant_name=sname,
                        wait_mode="sem-ge-imm", wait_value=val)]
                    ins.sync_info = si
                    fixed += 1
    assert fixed == len(wr_rd_pairs), (fixed, len(wr_rd_pairs))


def _host_params(gt_sym_axis, gd_sym_axis):
    B = gt_sym_axis.shape[0]
    gt = gt_sym_axis.astype(np.float32)
    gds = gd_sym_axis.astype(np.float32)
    prm = []
    for i in range(B):
        sx = gds[i, 0]
        sy = gds[i, 1]
        dx = np.float32(-10.0) * gt[i, 0]
        dy = np.float32(10.0) * gt[i, 1]
        dy1f = np.float32(np.floor(dy))
        dx1f = np.float32(np.floor(dx))
        dy1 = int(dy1f)
        dx1 = int(dx1f)
        fy = np.float32(dy - dy1f)
        fx = np.float32(dx - dx1f)
        pos = bool(dx > 0)
        one = np.float32(1.0)
        zero = np.float32(0.0)
        if pos:
            wa = (one, zero)
            wb = (-(one - fy), -fy)
            rb, cb = dy1, dx1
            jlo, jhi = 0, W - dx1 - 1
        else:
            wa = (-fy, -(one - fy))
            wb = (one, zero)
            rb, cb = dy1 + 1, dx1
            jlo, jhi = -dx1, W
        rows = H - dy1 - 1
        cols = (W - dx1 - 1) if pos else (W + dx1)
        m = max(abs(float(sx)), abs(float(sy)), 1e-30)
        a = np.float32(float(sy) / m)
        b = np.float32(float(sx) / m)
        off = rb * W + cb
        assert 1 <= off and off + (H - 1) * W + WLEN <= R
        assert 0 <= rb <= RPAD - 2 and -16 <= cb <= 16
        assert 0 <= jlo <= jhi <= W
        prm.append(dict(a=a, b=b, wa=wa, wb=wb, rb=rb, cb=cb,
                        cb0=one - fx, cb1=fx, jlo=jlo, jhi=jhi,
                        rows=rows, cols=cols, scale=m * m))
    return prm


def _band(w0, w1):
    """lhsT[k, m] = w0*d(k==m) + w1*d(k==m+1), out-row 127 zeroed."""
    mat = np.zeros((P, P), np.float16)
    idx = np.arange(P)
    mat[idx, idx] = np.float16(w0)
    mat[idx[1:], idx[:-1]] = np.float16(w1)
    mat[:, P - 1] = np.float16(0.0)
    return mat


def _seam_fix(grid_s, p):
    """Exact fp64 contribution of the device-zeroed rows 127/255/383/511."""
    g0 = grid_s[0].astype(np.float64)
    g1 = grid_s[1].astype(np.float64)
    G = p["a"] * g0 + p["b"] * g1
    Gp = np.vstack([G, np.zeros((RPAD, W))])
    flat = Gp.reshape(-1)
    wa0, wa1 = float(p["wa"][0]), float(p["wa"][1])
    wb0, wb1 = float(p["wb"][0]), float(p["wb"][1])
    cb0, cb1 = float(p["cb0"]), float(p["cb1"])
    base = p["rb"] * W + p["cb"]
    jlo, jhi = p["jlo"], p["jhi"]
    ssq = 0.0
    for r in (127, 255, 383, 511):
        if r >= p["rows"]:
            continue
        w_r = flat[base + r * W: base + r * W + W + 1]
        w_r1 = flat[base + (r + 1) * W: base + (r + 1) * W + W + 1]
        bc_r = cb0 * w_r[0:W] + cb1 * w_r[1:W + 1]
        bc_r1 = cb0 * w_r1[0:W] + cb1 * w_r1[1:W + 1]
        g_r1 = G[r + 1] if r + 1 < H else np.zeros(W)
        d = wa0 * G[r] + wa1 * g_r1 + wb0 * bc_r + wb1 * bc_r1
        ssq += float((d[jlo:jhi] ** 2).sum())
    return ssq


def kernel(grid, gt_sym_axis, gd_sym_axis):
    grid = np.ascontiguousarray(grid, dtype=np.float32)
    B = grid.shape[0]
    assert grid.shape == (B, 2, H, W) and B == NS * NCORES

    if "nc" not in _CACHE:
        nc = _build_program()
        _split_multiwaits(nc)
        _CACHE["nc"] = nc
    nc = _CACHE["nc"]

    prm = _host_params(np.asarray(gt_sym_axis), np.asarray(gd_sym_axis))

    i_of_pq = np.arange(H).reshape(Q, P).T
    in_maps = []
    for c in range(NCORES):
        pfv = np.zeros((P, NS * NPF), np.float32)
        piv = np.zeros((1, NS), np.int32)
        ivv = np.zeros((P, NS * Q), np.float16)
        matv = np.zeros((P, NS * 3 * P), np.float16)
        for s in range(NS):
            p = prm[c * NS + s]
            pfv[:, s * NPF + COL_A] = p["a"]
            pfv[:, s * NPF + COL_B] = p["b"]
            piv[0, s] = p["rb"] * W + p["cb"]
            ivv[:, s * Q:(s + 1) * Q] = (i_of_pq < p["rows"]).astype(np.float16)
            matv[:, (3 * s) * P:(3 * s + 1) * P] = _band(*p["wa"])
            bb = _band(*p["wb"])
            matv[:, (3 * s + 1) * P:(3 * s + 2) * P] = (
                bb * np.float16(p["cb0"]))
            matv[:, (3 * s + 2) * P:(3 * s + 3) * P] = (
                bb * np.float16(p["cb1"]))
        in_maps.append({
            "g": grid[c * NS:(c + 1) * NS],
            "pf": pfv, "pi": piv, "iv": ivv, "mats": matv,
        })

    res = run_bass_kernel_spmd(nc, in_maps, core_ids=list(range(NCORES)))

    losses = np.zeros(B, np.float64)
    for c in range(NCORES):
        o = res.results[c]["out"]
        for s in range(NS):
            p = prm[c * NS + s]
            ssq = float(o[s, p["jlo"]:p["jhi"]].sum(dtype=np.float64))
            ssq += _seam_fix(grid[c * NS + s], p)
            count = float(np.float32(p["rows"] * p["cols"]))
            losses[c * NS + s] = p["scale"] * ssq / count
    return np.float32(losses.mean())
